# revision 1
# baseline (speedup 1.0000x reference)
"""GAT forward (2-layer graph attention, B=4 N=2048 F=128 H=8 D=64 C=32)
as a Bass/Tile SPMD kernel on 8 Trainium2 NeuronCores.

Sharding: core c -> (batch b=c//2, query-row half c%2).  Each core computes
attention for its 1024 query rows over all 2048 keys for all 8 heads
(layer 1) and for the output head (layer 2).  The only cross-core exchange
is a 2-rank AllGather of the layer-2 projections [g1|g2|Wh2] ([1024,34] f32)
within each (2b, 2b+1) pair.

Layout: attention logits are built TRANSPOSED, e^T[j (keys) = partitions,
i (queries) = free], so the PV matmul needs no operand transposes
(lhsT = Wh[j,d] stationary, rhs = p[j,i] moving, out = h^T[d,i]) and
softmax row sums come from a ones-column appended to Wh (PSUM row D).

The N^2 elementwise work (mask+f1+f2, leaky-relu, exp over 144 [128,1024]
logit tiles) is the bottleneck; everything runs in bf16 (2x DVE tensor-
tensor throughput; ACT is dtype-independent) and every logit PAIR of tiles
is routed down one of three pipelines to saturate ACT+DVE+GPSIMD jointly:

  P3 : u = mb + f1rep       (DVE tensor_tensor, bf16 2x)
       z = Prelu(u + f2col) (ACT, per-sub bias)     p = Exp(z)  (ACT)
  P4 : same but the combine runs on GPSIMD tensor_add
  P2 : STT combine (incl f2) + DVE prelu; exp is the only ACT op
  P7 : exp(prelu(u)) == max(exp(u), exp(0.2u)); exps factor rank-1 and,
       after a softmax-invariant per-query rescale by exp(-0.2 f1):
       q1 = exp(.8 f1_i)*exp(f2_j) + mb     (DVE STT mult,add)
       q2 = exp(.2 f2_j) + mb               (DVE tensor-scalar add)
       p  = relu(max(q1, q2))               (DVE bf16 2x ops; no ACT!)
       (mb = -1e15 masked / 0 unmasked dominates both branches exactly)

Softmax division is deferred to the per-head-pair epilogue (1/S via Ln/Exp
of the row-sum), fused with ELU via elu(v)+1 = relu(v) + exp(min(v,0)),
the +1 folded into a rank-1 correction matmul of the layer-2 projection.
"""

import numpy as np
import ml_dtypes

import concourse.bass as bass
import concourse.tile as tile
from concourse import mybir
from concourse.bass_utils import run_bass_kernel_spmd

F32 = mybir.dt.float32
F32R = mybir.dt.float32r
BF16 = mybir.dt.bfloat16

B, N, F, H, D, C = 4, 2048, 128, 8, 64, 32
I = N // 2          # query rows per core
JT = N // 128       # key tiles
IC = I // 128       # query-row 128-chunks per core
KT = (H * D) // 128 # hidden-dim 128-chunks
ALPHA = 0.2
BIG = 1e15          # mask bias; also dominates A*B in the factored path
N_CORES = 8
REPLICA_GROUPS = [[0, 1], [2, 3], [4, 5], [6, 7]]

ADD = mybir.AluOpType.add
MAX = mybir.AluOpType.max
MULT = mybir.AluOpType.mult
# NOTE: hardware "Lrelu" has a fixed 0.01 slope and ignores alpha;
# "Prelu" honors alpha (verified on HW) — it is the configurable leaky relu.
ACT_LRELU = mybir.ActivationFunctionType.Prelu
ACT_EXP = mybir.ActivationFunctionType.Exp
ACT_LN = mybir.ActivationFunctionType.Ln
ACT_COPY = mybir.ActivationFunctionType.Copy

N_HEADS_ALL = H + 1          # 8 layer-1 heads + the layer-2 output head
PAIRS_PER_HEAD = JT // 2     # 8
N_PAIRS = N_HEADS_ALL * PAIRS_PER_HEAD  # 72


def _split_multiwaits(nc):
    """Pinned walrus accepts only one sync-wait per instruction; Tile's exit
    drain (and occasionally others) carries several.  Hoist extras onto
    single-wait Drains on the same engine immediately before the owner."""
    n_fixed = 0
    for fn in nc.m.functions:
        for bb in fn.blocks:
            for name in [i.name for i in bb.instructions]:
                idx = [i.name for i in bb.instructions].index(name)
                inst = bb.instructions[idx]
                si = inst.sync_info
                if si is None or len(si.on_wait) <= 1:
                    continue
                waits = list(si.on_wait)
                for k, w in enumerate(waits[:-1]):
                    nd = mybir.InstDrain(
                        name=f"waitfix-{inst.name}-{k}", ins=[], outs=[])
                    nd.engine = inst.engine
                    nd.sync_info = mybir.SyncInfo(on_wait=[w], on_update=[])
                    nc.register_instruction(nd, overwrite=True)
                    bb.instructions.insert(idx + k, nd)
                inst.sync_info = mybir.SyncInfo(
                    on_wait=waits[-1:], on_update=list(si.on_update))
                n_fixed += 1
    return n_fixed


def _make_route(cfg):
    """Per-pair pipeline assignment:
      3 (DVE-add + ACT prelu/exp), 4 (GPS-add + ACT prelu/exp),
      7 (factored rank-1 exps on DVE + GPS max, no ACT),
      2 (DVE STT-add + DVE prelu + ACT exp only).
    Counts spread across the 9 'heads' (8 L1 + L2); p7 optionally
    concentrated into few heads (fewer exp(f1)-prep ops on ACT)."""
    if "route" in cfg:                      # explicit per-pair override
        route = list(cfg["route"])
        assert len(route) == N_PAIRS
        return route
    n4 = int(cfg.get("p4", 24))
    n7 = int(cfg.get("p7", 33))
    n2 = int(cfg.get("p2", 0))
    n3 = N_PAIRS - n4 - n7 - n2
    assert n3 >= 0
    conc = int(cfg.get("p7_conc", 0))  # 0 = spread; 1 = concentrate P7
    per_head = [[0, 0, 0, 0] for _ in range(N_HEADS_ALL)]  # [n3, n4, n7, n2]
    if conc:
        # fill P7 into heads round-robin starting at head 1, PAIRS_PER_HEAD
        # per head; distribute the rest evenly over remaining slots
        order = [1, 3, 5, 7, 8, 0, 2, 4, 6]
        left7 = n7
        for hh in order:
            take = min(left7, PAIRS_PER_HEAD)
            per_head[hh][2] = take
            left7 -= take
            if left7 == 0:
                break
        slots = [PAIRS_PER_HEAD - per_head[hh][2] for hh in range(N_HEADS_ALL)]
        for idx, cnt in ((1, n4), (3, n2), (0, n3)):
            left = cnt
            while left > 0:
                done = True
                for hh in range(N_HEADS_ALL):
                    used = sum(per_head[hh])
                    if used < PAIRS_PER_HEAD and left > 0:
                        per_head[hh][idx] += 1
                        left -= 1
                        done = False
                if done:
                    break
    else:
        for idx, cnt in enumerate((n3, n4, n7, n2)):
            base, rem = divmod(cnt, N_HEADS_ALL)
            for hh in range(N_HEADS_ALL):
                per_head[hh][idx] = base + (1 if hh < rem else 0)
        for hh in range(N_HEADS_ALL):
            while sum(per_head[hh]) > PAIRS_PER_HEAD:
                per_head[hh][int(np.argmax(per_head[hh]))] -= 1
            while sum(per_head[hh]) < PAIRS_PER_HEAD:
                per_head[hh][int(np.argmin(per_head[hh]))] += 1
    if "l2route" in cfg:
        l2r = list(cfg["l2route"])
        assert len(l2r) == PAIRS_PER_HEAD
    else:
        l2r = None
    route = []
    for hh in range(N_HEADS_ALL):
        if hh == H and l2r is not None:
            route.extend(l2r)
            continue
        c3, c4, c7, c2 = per_head[hh]
        pool = [7] * c7 + [4] * c4 + [3] * c3 + [2] * c2
        mix, lo, hi = [], 0, len(pool) - 1
        toggle = True
        while lo <= hi:
            if toggle:
                mix.append(pool[lo]); lo += 1
            else:
                mix.append(pool[hi]); hi -= 1
            toggle = not toggle
        route.extend(mix)
    return route


def build_program(with_collective=True, cfg=None, repeat=1):
    cfg = dict(cfg or {})
    QB[0] = int(cfg.get("qbufs", 3))
    route = _make_route(cfg)

    nc = bass.Bass("TRN2", target_bir_lowering=False, debug=False,
                   enable_asserts=False, num_devices=N_CORES)

    xt_d = nc.dram_tensor("xt", [F, N], BF16, kind="ExternalInput")
    xtl_d = nc.dram_tensor("xtl", [F, I], BF16, kind="ExternalInput")
    mb_d = nc.dram_tensor("mb", [JT, 128, I], BF16, kind="ExternalInput")
    wext_d = nc.dram_tensor("wext", [H, F, D + 2], BF16, kind="ExternalInput")
    a1rep_d = nc.dram_tensor("a1rep", [H, F, 128], BF16, kind="ExternalInput")
    woext_d = nc.dram_tensor("woext", [KT, 128, C + 2], F32,
                             kind="ExternalInput")
    wcorr_d = nc.dram_tensor("wcorr", [1, C + 2], F32, kind="ExternalInput")
    ident_d = nc.dram_tensor("ident", [128, 128], F32, kind="ExternalInput")
    outp_d = nc.dram_tensor("outp", [I, C], F32, kind="ExternalOutput")

    with tile.TileContext(nc) as tc:
        if repeat > 1:
            def body(iv, unroll=None):
                _build_body(nc, tc, xt_d, xtl_d, mb_d, wext_d, a1rep_d,
                            woext_d, wcorr_d, ident_d, outp_d,
                            with_collective, route, cfg)
            with tc.For_i(0, repeat, 1) as iv:
                body(iv)
        else:
            _build_body(nc, tc, xt_d, xtl_d, mb_d, wext_d, a1rep_d,
                        woext_d, wcorr_d, ident_d, outp_d,
                        with_collective, route, cfg)
    _split_multiwaits(nc)
    return nc


QB = [3]


def _emit_pair(nc, work, workp, mode, subs, pair_args):
    """Emit one logit pair.  subs = [(jt, mb_ap, mq_ap, f2col_ap, e1col_ap,
    e2col_ap)]; pair_args = (f1rep, B1, B2)."""
    f1rep, B1, B2 = pair_args
    if mode == 7:
        q1 = work.tile([128, 2, I], BF16, tag="q1", bufs=QB[0])
        q2 = work.tile([128, 2, I], BF16, tag="q2", bufs=QB[0])
        for k, (jt, mb_ap, f2c, e1c, e2c) in enumerate(subs):
            nc.vector.scalar_tensor_tensor(
                out=q1[:, k, :], in0=B1[:], scalar=e1c, in1=mb_ap,
                op0=MULT, op1=ADD)
            nc.vector.tensor_scalar_add(q2[:, k, :], mb_ap, e2c)
        p = workp.tile([128, 2, I], BF16, tag="p")
        nc.vector.tensor_tensor(out=p[:], in0=q1[:], in1=q2[:], op=MAX)
        nc.vector.tensor_scalar_max(p[:], p[:], 0.0)
        return p
    if mode == 2:
        # combine WITH f2 (STT) so prelu can run on DVE; exp is the only ACT op
        u = work.tile([128, 2, I], BF16, tag="u")
        for k, (jt, mb_ap, f2c, e1c, e2c) in enumerate(subs):
            nc.vector.scalar_tensor_tensor(
                out=u[:, k, :], in0=mb_ap, scalar=f2c, in1=f1rep[:],
                op0=ADD, op1=ADD)
        w = work.tile([128, 2, I], BF16, tag="q1", bufs=QB[0])
        nc.vector.tensor_scalar_mul(w[:], u[:], ALPHA)     # bf16 4x
        nc.vector.tensor_tensor(out=u[:], in0=u[:], in1=w[:], op=MAX)
        p = workp.tile([128, 2, I], BF16, tag="p")
        nc.scalar.activation(p[:], u[:], ACT_EXP)
        return p
    u = work.tile([128, 2, I], BF16, tag="u")
    for k, (jt, mb_ap, f2c, e1c, e2c) in enumerate(subs):
        if mode == 4:
            nc.gpsimd.tensor_add(u[:, k, :], mb_ap, f1rep[:])
        else:
            nc.vector.tensor_tensor(out=u[:, k, :], in0=mb_ap, in1=f1rep[:],
                                    op=ADD)
    for k, (jt, mb_ap, f2c, e1c, e2c) in enumerate(subs):
        nc.scalar.activation(u[:, k, :], u[:, k, :], ACT_LRELU,
                             bias=f2c, alpha=ALPHA)
    p = workp.tile([128, 2, I], BF16, tag="p")
    nc.scalar.activation(p[:], u[:], ACT_EXP)
    return p


def _copy_engine(nc, eng, out, in_):
    if eng == "act":
        nc.scalar.activation(out, in_, ACT_COPY)
    elif eng == "gps":
        nc.gpsimd.tensor_copy(out=out, in_=in_)
    else:
        nc.vector.tensor_copy(out=out, in_=in_)


def _build_body(nc, tc, xt_d, xtl_d, mb_d, wext_d, a1rep_d, woext_d,
                wcorr_d, ident_d, outp_d, with_collective, route, cfg):
    from contextlib import ExitStack
    ctx = ExitStack()
    f1rep_eng = cfg.get("f1rep_eng", "dve")
    rbc_eng = cfg.get("rbc_eng", "act")
    assert f1rep_eng != "gps" and rbc_eng != "gps"  # GPSIMD cannot read PSUM
    
    ph0_engs = cfg.get("ph0_engs", ("act", "dve"))
    ep_v_gps = False  # GPSIMD cannot read PSUM (hT)
    with ctx:
        singles = ctx.enter_context(tc.tile_pool(name="singles", bufs=1))
        psA = ctx.enter_context(tc.tile_pool(name="psA", bufs=2, space="PSUM"))
        psB = ctx.enter_context(tc.tile_pool(name="psB", bufs=1, space="PSUM"))
        psC = ctx.enter_context(tc.tile_pool(name="psC", bufs=2, space="PSUM"))
        dram = ctx.enter_context(tc.tile_pool(name="dram", bufs=1,
                                              space="DRAM"))

        # ---------------- persistent loads ----------------
        mb_s = singles.tile([128, JT, I], BF16)
        nc.sync.dma_start(out=mb_s[:, 0:2, :],
                          in_=mb_d.ap()[0:2].rearrange("jt p i -> p jt i"))
        xtl_s = singles.tile([F, I], BF16)
        nc.sync.dma_start(out=xtl_s[:], in_=xtl_d.ap())
        a1rep_s = singles.tile([F, H, 128], BF16)
        nc.sync.dma_start(out=a1rep_s[:],
                          in_=a1rep_d.ap().rearrange("h f e -> f h e"))
        wcorr_s = singles.tile([1, C + 2], F32)
        nc.sync.dma_start(out=wcorr_s[:], in_=wcorr_d.ap())
        ident_s = singles.tile([128, 128], F32)
        nc.sync.dma_start(out=ident_s[:], in_=ident_d.ap())
        woext_raw = singles.tile([128, KT, C + 2], F32)
        nc.sync.dma_start(out=woext_raw[:],
                          in_=woext_d.ap().rearrange("k f e -> f k e"))
        woext_s = singles.tile([128, KT, C + 2], F32R)
        nc.vector.tensor_copy(out=woext_s[:], in_=woext_raw[:])

        ones_s = singles.tile([1, 128], BF16)
        nc.gpsimd.memset(ones_s[:], 1.0)
        onesf_s = singles.tile([1, 128], F32)
        nc.gpsimd.memset(onesf_s[:], 1.0)

        whbuf = singles.tile([128, H, JT, D + 1], BF16)
        nc.gpsimd.memset(whbuf[:, :, :, D:D + 1], 1.0)
        fcol = singles.tile([128, H, JT, 1], F32)
        fexp1 = singles.tile([128, H, JT, 1], F32)
        fexp2 = singles.tile([128, H, JT, 1], F32)
        hcatT = singles.tile([128, KT, I], F32R)

        xt_s = singles.tile([F, N], BF16)
        nc.sync.dma_start(out=xt_s[:], in_=xt_d.ap())
        wext_s = singles.tile([F, H, D + 2], BF16)
        nc.sync.dma_start(out=wext_s[:],
                          in_=wext_d.ap().rearrange("h f e -> f h e"))
        for j0 in (2, 6, 10):
            j1 = j0 + 4 if j0 < 10 else JT
            nc.sync.dma_start(
                out=mb_s[:, j0:j1, :],
                in_=mb_d.ap()[j0:j1].rearrange("jt p i -> p jt i"))

        work = ctx.enter_context(tc.tile_pool(name="work", bufs=4))
        workp = ctx.enter_context(tc.tile_pool(name="workp", bufs=4))
        ep1 = ctx.enter_context(tc.tile_pool(name="ep1", bufs=1))
        ep2 = ctx.enter_context(tc.tile_pool(name="ep2", bufs=2))
        epL2 = ctx.enter_context(tc.tile_pool(name="epL2", bufs=1))

        def emit_phase0_head(h):
            # Wh tiles + f columns for head h (emitted per-head so the
            # copies overlap the previous head's logit work)
            for jg in range(JT // 4):
                whp = psA.tile([128, 4, D + 2], F32, tag="ph")
                for k in range(4):
                    jt = jg * 4 + k
                    nc.tensor.matmul(whp[:, k, :],
                                     lhsT=xt_s[:, jt * 128:(jt + 1) * 128],
                                     rhs=wext_s[:, h, :])
                dst = whbuf[:, h, jg * 4:(jg + 1) * 4, 0:D]
                _copy_engine(nc, ph0_engs[jg % len(ph0_engs)],
                             dst, whp[:, :, 0:D])
                nc.vector.tensor_copy(
                    out=fcol[:, h, jg * 4:(jg + 1) * 4, :],
                    in_=whp[:, :, D + 1:D + 2])
            nc.scalar.activation(fexp1[:, h], fcol[:, h], ACT_EXP)
            nc.scalar.activation(fexp2[:, h], fcol[:, h], ACT_EXP,
                                 scale=ALPHA)

        ep_state = {}

        def emit_half_ep(hT, h):
            # per-head half-epilogue: rinv = 1/S via DVE reciprocal, SWDGE
            # partition broadcast, v-half = hT*rinv.  The odd half finishes:
            # hcat = elu(v)+1 = relu(v)+exp(min(v,0)).
            rinv = ep1.tile([1, I], F32, tag=f"ri{h % 2}", bufs=1)
            if cfg.get("dbg_ep_lnexp"):
                lnS_ = ep1.tile([1, I], F32, tag=f"ln{h % 2}", bufs=2)
                nc.scalar.activation(lnS_[:], hT[D:D + 1, :], ACT_LN)
                nc.scalar.activation(rinv[:], lnS_[:], ACT_EXP, scale=-1.0)
            else:
                nc.vector.reciprocal(rinv[:], hT[D:D + 1, :])
            if h % 2 == 0:
                v = ep1.tile([128, I], BF16, tag="v", bufs=2)
                ep_state["v"] = v
                half = slice(0, D)
            else:
                v = ep_state["v"]
                half = slice(D, 128)
            rbp = psB.tile([128, I], F32, tag="rep")
            for hf_ in range(I // 512):
                sl_ = slice(hf_ * 512, (hf_ + 1) * 512)
                nc.tensor.matmul(rbp[0:D, sl_], lhsT=onesf_s[0:1, 0:D],
                                 rhs=rinv[0:1, sl_])
            # rbc half kept at base partition 0: DVE inputs must share bases
            rbc = ep1.tile([D, I], F32, tag=f"rb{h % 2}", bufs=1)
            _copy_engine(nc, rbc_eng, rbc[:], rbp[0:D, :])
            nc.vector.tensor_tensor(out=v[half, :], in0=hT[0:D, :],
                                    in1=rbc[:], op=MULT)
            if h % 2 == 1:
                t = ep1.tile([128, I], BF16, tag="t", bufs=2)
                nc.vector.tensor_scalar_min(t[:], v[:], 0.0)
                nc.scalar.activation(t[:], t[:], ACT_EXP)
                nc.vector.scalar_tensor_tensor(
                    out=hcatT[:, h // 2, :], in0=v[:], scalar=0.0, in1=t[:],
                    op0=MAX, op1=ADD)

        def emit_head_prep(h):
            emit_phase0_head(h)
            head_modes = route[h * PAIRS_PER_HEAD:(h + 1) * PAIRS_PER_HEAD]
            need_f1rep = any(m in (2, 3, 4) for m in head_modes)
            need_B = any(m == 7 for m in head_modes)
            f1p = psB.tile([128, I], F32, tag="rep")
            for hf in range(I // 512):
                sl = slice(hf * 512, (hf + 1) * 512)
                nc.tensor.matmul(f1p[:, sl], lhsT=a1rep_s[:, h, :],
                                 rhs=xtl_s[:, sl])
            f1rep_s = B1_s = None
            B2_s = True  # unused (q2 needs only the per-partition scalar)
            if need_f1rep:
                f1rep_s = ep2.tile([128, I], BF16, tag="f1rep")
                _copy_engine(nc, f1rep_eng, f1rep_s[:], f1p[:])
            if need_B:
                B1_s = ep2.tile([128, I], BF16, tag="B1")
                nc.scalar.activation(B1_s[:], f1p[:], ACT_EXP, scale=1.0 - ALPHA)
            return f1rep_s, B1_s, B2_s

        # ---------------- layer 1 ----------------
        pending_ep = None   # deferred half-epilogue (software pipelining)
        preps = emit_head_prep(0)
        for h in range(H):
            head_modes = route[h * PAIRS_PER_HEAD:(h + 1) * PAIRS_PER_HEAD]
            cur = preps
            hT = psC.tile([D + 1, I], F32, tag="acc")
            for jp in range(PAIRS_PER_HEAD):
                if jp == int(cfg.get("ep_defer", 2)) and pending_ep is not None:
                    pending_ep()
                    pending_ep = None
                if jp == int(cfg.get("prep_at", 4)) and h + 1 < H:
                    preps = emit_head_prep(h + 1)
                mode = head_modes[jp]
                subs = []
                for k in range(2):
                    jt = jp * 2 + k
                    subs.append((jt, mb_s[:, jt, :],
                                 fcol[:, h, jt, :], fexp1[:, h, jt, :],
                                 fexp2[:, h, jt, :]))
                p = _emit_pair(nc, work, workp, mode, subs, cur)
                for k in range(2):
                    jt = jp * 2 + k
                    for hf in range(I // 512):
                        sl = slice(hf * 512, (hf + 1) * 512)
                        nc.tensor.matmul(hT[:, sl],
                                         lhsT=whbuf[:, h, jt, :],
                                         rhs=p[:, k, sl],
                                         start=(jt == 0), stop=(jt == JT - 1))

            pending_ep = (lambda t_=hT, h_=h: emit_half_ep(t_, h_))
        if pending_ep is not None:
            pending_ep()
            pending_ep = None

        # ---------------- layer 2 projection + gather (bf16 payload) ------
        wh2loc = singles.tile([128, IC, C + 2], F32)
        gin = dram.tile([I, C + 2], F32)
        for ic in range(IC):
            w2p = psA.tile([128, 4, D + 2], F32, tag="ph")
            for kt in range(KT):
                nc.tensor.matmul(
                    w2p[:, 0, 0:C + 2],
                    lhsT=hcatT[:, kt, ic * 128:(ic + 1) * 128],
                    rhs=woext_s[:, kt, :],
                    start=(kt == 0), stop=False)
            nc.tensor.matmul(w2p[:, 0, 0:C + 2], lhsT=onesf_s[0:1, :],
                             rhs=wcorr_s[:], start=False, stop=True)
            nc.vector.tensor_copy(out=wh2loc[:, ic, :], in_=w2p[:, 0, 0:C + 2])
        nc.sync.dma_start(
            out=gin.rearrange("(ic p) c -> p ic c", p=128),
            in_=wh2loc[:])

        gout = dram.tile([N, C + 2], F32)
        if with_collective:
            nc.gpsimd.collective_compute(
                "AllGather", mybir.AluOpType.bypass,
                replica_groups=REPLICA_GROUPS,
                ins=[gin.opt()], outs=[gout.opt()])
        else:  # timing-model variant: fake the exchange with two local copies
            nc.sync.dma_start(out=gout[0:I, :], in_=gin[:])
            nc.sync.dma_start(out=gout[I:N, :], in_=gin[:])

        # g1 row (local queries) -> replicated [128, I]
        g1rowp = psB.tile([128, I], F32, tag="rep")
        for ic in range(IC):
            nc.tensor.transpose(g1rowp[0:1, ic * 128:(ic + 1) * 128],
                                in_=wh2loc[:, ic, 0:1], identity=ident_s[:])
        g1row_s = epL2.tile([1, I], BF16, tag="g1row")
        nc.scalar.activation(g1row_s[:], g1rowp[0:1, :], ACT_COPY)
        g1rp = psB.tile([128, I], F32, tag="rep")
        for hf in range(I // 512):
            sl = slice(hf * 512, (hf + 1) * 512)
            nc.tensor.matmul(g1rp[:, sl], lhsT=ones_s[0:1, :],
                             rhs=g1row_s[0:1, sl])
        l2_modes = route[H * PAIRS_PER_HEAD:]
        g1rep_s = B1L2 = None
        B2L2 = True
        if any(m in (2, 3, 4) for m in l2_modes):
            g1rep_s = singles.tile([128, I], BF16)
            nc.vector.tensor_copy(out=g1rep_s[:], in_=g1rp[:])
        if any(m == 7 for m in l2_modes):
            B1L2 = singles.tile([128, I], BF16)
            nc.scalar.activation(B1L2[:], g1rp[:], ACT_EXP, scale=1.0 - ALPHA)

        # gathered rows: [g1, g2, Wh2(32)] f32 staged, bf16 for the PV lhsT
        wh2tmp = singles.tile([128, JT, C + 2], F32)
        nc.sync.dma_start(
            out=wh2tmp[:],
            in_=gout.rearrange("(jt p) c -> p jt c", p=128))
        wh2gr = singles.tile([128, JT, C + 3], BF16)
        nc.gpsimd.memset(wh2gr[:, :, C + 2:C + 3], 1.0)
        its1 = singles.tile([128, JT, 1], F32)
        its2 = singles.tile([128, JT, 1], F32)
        for jg in range(JT // 4):
            s4 = slice(jg * 4, (jg + 1) * 4)
            nc.gpsimd.tensor_copy(out=wh2gr[:, s4, 0:C + 2],
                                  in_=wh2tmp[:, s4, :])
            nc.scalar.activation(its1[:, s4, :], wh2tmp[:, s4, 1:2], ACT_EXP)
            nc.scalar.activation(its2[:, s4, :], wh2tmp[:, s4, 1:2], ACT_EXP,
                                 scale=ALPHA)

        # ---------------- layer 2 attention ----------------
        # hoist the gather-independent combines (mask + g1rep) so DVE/GPS
        # work while the AllGather is still in flight; reuse the idle q1/q2
        # rings so the main u-ring keeps flowing
        l2_u = {}
        hoist_tags = ["q1", "q1", "q2", "q2"]
        for jp in range(PAIRS_PER_HEAD):
            if l2_modes[jp] not in (3, 4) or not hoist_tags:
                continue
            u = work.tile([128, 2, I], BF16, tag=hoist_tags.pop(0), bufs=QB[0])
            for k in range(2):
                jt = jp * 2 + k
                if l2_modes[jp] == 4:
                    nc.gpsimd.tensor_add(u[:, k, :], mb_s[:, jt, :],
                                         g1rep_s[:])
                else:
                    nc.vector.tensor_tensor(out=u[:, k, :],
                                            in0=mb_s[:, jt, :],
                                            in1=g1rep_s[:], op=ADD)
            l2_u[jp] = u

        o2T = psC.tile([D + 1, I], F32, tag="acc")
        for jp in range(PAIRS_PER_HEAD):
            mode = l2_modes[jp]
            subs = []
            for k in range(2):
                jt = jp * 2 + k
                subs.append((jt, mb_s[:, jt, :],
                             wh2tmp[:, jt, 1:2], its1[:, jt, :],
                             its2[:, jt, :]))
            if jp in l2_u:
                u = l2_u[jp]
                for k, (jt, mb_ap, f2c, e1c, e2c) in enumerate(subs):
                    nc.scalar.activation(u[:, k, :], u[:, k, :], ACT_LRELU,
                                         bias=f2c, alpha=ALPHA)
                p = workp.tile([128, 2, I], BF16, tag="p")
                nc.scalar.activation(p[:], u[:], ACT_EXP)
            else:
                p = _emit_pair(nc, work, workp, mode, subs,
                               (g1rep_s, B1L2, B2L2))
            for k in range(2):
                jt = jp * 2 + k
                for hf in range(I // 512):
                    sl = slice(hf * 512, (hf + 1) * 512)
                    nc.tensor.matmul(o2T[0:C + 1, sl],
                                     lhsT=wh2gr[:, jt, 2:C + 3],
                                     rhs=p[:, k, sl],
                                     start=(jt == 0), stop=(jt == JT - 1))

        # ---------------- finalize (transposed: per-query reciprocal) -----
        if cfg.get("dbg_simple_fin"):
            r2ln = epL2.tile([1, I], F32, tag="lnS2")
            nc.scalar.activation(r2ln[:], o2T[C:C + 1, :], ACT_LN)
            r2 = epL2.tile([1, I], BF16, tag="r2")
            nc.scalar.activation(r2[:], r2ln[:], ACT_EXP, scale=-1.0)
            rbc2p = psB.tile([128, I], F32, tag="rep")
            for hf in range(I // 512):
                sl = slice(hf * 512, (hf + 1) * 512)
                nc.tensor.matmul(rbc2p[0:C, sl], lhsT=ones_s[0:1, 0:C],
                                 rhs=r2[0:1, sl])
            rbc2_s = epL2.tile([C, I], F32, tag="rbc2")
            nc.vector.tensor_copy(out=rbc2_s[:], in_=rbc2p[0:C, :])
            oT_s = epL2.tile([C, I], F32, tag="oT")
            nc.vector.tensor_tensor(out=oT_s[:], in0=o2T[0:C, :],
                                    in1=rbc2_s[:], op=MULT)
            for k in range(IC):
                ofp = psA.tile([128, 4, D + 2], F32, tag="ph")
                nc.tensor.transpose(ofp[:, 0, 0:C],
                                    in_=oT_s[:, k * 128:(k + 1) * 128],
                                    identity=ident_s[0:C, 0:C])
                ofs = ep2.tile([128, C], F32, tag="ofs")
                nc.vector.tensor_copy(out=ofs[:], in_=ofp[:, 0, 0:C])
                nc.sync.dma_start(out=outp_d.ap()[k * 128:(k + 1) * 128, :],
                                  in_=ofs[:])
        else:
            o2s = epL2.tile([C, I], F32, tag="o2s")
            nc.vector.tensor_copy(out=o2s[:], in_=o2T[0:C, :])
            o2r = epL2.tile([1, I], F32, tag="o2r")
            nc.scalar.activation(o2r[:], o2T[C:C + 1, :], ACT_COPY)
            ofs_all = epL2.tile([128, IC, C], F32, tag="ofs_all")
            for k in range(IC):
                ck = slice(k * 128, (k + 1) * 128)
                ofp = psA.tile([128, 4, D + 2], F32, tag="ph")
                nc.tensor.transpose(ofp[:, 0, 0:C], in_=o2s[:, ck],
                                    identity=ident_s[0:C, 0:C])
                ofq = psA.tile([128, 4, D + 2], F32, tag="ph")
                nc.tensor.transpose(ofq[:, 0, 0:1], in_=o2r[:, ck],
                                    identity=ident_s[0:1, 0:1])
                s2t = ep2.tile([128, 1], F32, tag="s2t")
                nc.vector.reciprocal(s2t[:], ofq[:, 0, 0:1])
                nc.vector.tensor_scalar_mul(ofs_all[:, k, :],
                                            ofp[:, 0, 0:C], s2t[:])
            nc.sync.dma_start(
                out=outp_d.ap().rearrange("(k p) c -> p k c", p=128),
                in_=ofs_all[:])


# --------------------------------------------------------------------------
# host side
# --------------------------------------------------------------------------

def shard_inputs(x, adj, W, a1, a2, Wo, ao1, ao2):
    x = np.asarray(x, np.float32)
    adj = np.asarray(adj)
    W = np.asarray(W, np.float32)
    a1 = np.asarray(a1, np.float32)
    a2 = np.asarray(a2, np.float32)
    Wo = np.asarray(Wo, np.float32)
    ao1 = np.asarray(ao1, np.float32)
    ao2 = np.asarray(ao2, np.float32)
    BF = ml_dtypes.bfloat16

    wvec1 = np.einsum("hfd,hd->hf", W, a1)          # [H, F]
    wvec2 = np.einsum("hfd,hd->hf", W, a2)
    wext = np.concatenate([W, wvec1[:, :, None], wvec2[:, :, None]],
                          axis=2).astype(BF)
    a1rep = np.repeat(wvec1[:, :, None], 128, axis=2).astype(BF)
    wo1 = Wo @ ao1                                   # [512]
    wo2 = Wo @ ao2
    woflat = np.concatenate([wo1[:, None], wo2[:, None], Wo], 1)  # [512, 34]
    woext = woflat.reshape(KT, 128, C + 2).astype(np.float32)
    wcorr = (-woflat.sum(0))[None, :].astype(np.float32)
    ident = np.eye(128, dtype=np.float32)

    in_maps = []
    for c in range(N_CORES):
        b, half = c // 2, c % 2
        i0 = half * I
        xt = np.ascontiguousarray(x[b].T).astype(BF)   # [F, N]
        xtl = np.ascontiguousarray(xt[:, i0:i0 + I])
        adjt = adj[b, i0:i0 + I, :].T                # [N, I] = (j, i)
        mb = np.where(adjt > 0, np.float32(0.0), np.float32(-BIG))
        mb = np.ascontiguousarray(mb.reshape(JT, 128, I)).astype(BF)
        in_maps.append({
            "xt": xt, "xtl": xtl, "mb": mb, "wext": wext,
            "a1rep": a1rep, "woext": woext, "wcorr": wcorr, "ident": ident,
        })
    return in_maps


# Engine routing chosen by cost-model sweep (TimelineSim): 44 GPS-combine
# pairs + 27 factored all-DVE pairs + 1 DVE-combine pair in layer 1;
# layer-2 mix tuned separately.  Modeled ~256us/core; engine busy
# ACT ~204 / DVE ~190 / GPS ~177 / PE ~105 us.
DEFAULT_CFG = {"ep_defer": 2, "f1rep_eng": "dve", "rbc_eng": "dve",
               "prep_at": 3,
               # per-pair pipeline assignment found by TimelineSim
               # hill-climb (64 L1 pairs + 8 L2 pairs)
               "route": [3, 4, 7, 4, 4, 4, 7, 2, 7, 3, 7, 4, 4, 4, 7, 4, 7, 4, 7, 7, 4, 4, 4, 4, 7, 3, 7, 3, 7, 4, 4, 3, 7, 7, 3, 4, 7, 4, 7, 4, 7, 4, 4, 3, 7, 4, 4, 7, 4, 3, 7, 4, 7, 4, 4, 4, 7, 7, 3, 4, 7, 7, 4, 4, 4, 4, 7, 4, 7, 2, 4, 7]}

_CACHE = {}


def _program():
    if "nc" not in _CACHE:
        _CACHE["nc"] = build_program(with_collective=True, cfg=DEFAULT_CFG)
    return _CACHE["nc"]


def kernel(**inputs):
    nc = _program()
    in_maps = shard_inputs(**inputs)
    res = run_bass_kernel_spmd(nc, in_maps, list(range(N_CORES)))
    _CACHE["last_results"] = res
    out = np.empty((B, N, C), np.float32)
    for c in range(N_CORES):
        b, half = c // 2, c % 2
        out[b, half * I:(half + 1) * I, :] = res.results[c]["outp"]
    return out



# revision 31
# speedup vs baseline: 1.3632x; 1.3632x over previous
"""GAT forward (2-layer graph attention, B=4 N=2048 F=128 H=8 D=64 C=32)
as a Bass/Tile SPMD kernel on 8 Trainium2 NeuronCores.

Sharding: core c -> (batch b=c//2, query-row half c%2).  Each core computes
attention for its 1024 query rows over all 2048 keys for all 8 heads
(layer 1) and for the output head (layer 2).  The only cross-core exchange
is a 2-rank AllGather of the layer-2 projections [g1|g2|Wh2] ([1024,34] f32)
within each (2b, 2b+1) pair.

Layout: attention logits are built TRANSPOSED, e^T[j (keys) = partitions,
i (queries) = free], so the PV matmul needs no operand transposes
(lhsT = Wh[j,d] stationary, rhs = p[j,i] moving, out = h^T[d,i]) and
softmax row sums come from a ones-column appended to Wh (PSUM row D).

The N^2 elementwise work over 144 [128,1024] logit tiles is the bottleneck.
Key identity: after a softmax-invariant per-query rescale by exp(-.2 f1_i),
  exp(prelu(f1_i + f2_j)) = max(exp(.8 f1_i)*exp(f2_j), exp(.2 f2_j))
and the adjacency mask is applied MULTIPLICATIVELY (m01 in {0,1}), so a
pair of logit tiles [128,2,1024] needs only:
  P7 : q = ts(B1, e1c, e2c, mult, max)  per sub   (DVE tensor-scalar 4x!)
       p = q * m01                      per pair  (DVE tensor-tensor 2x)
  P5 : same q on DVE; p = q * m01 on GPSIMD (tensor_mult)
  P3 : u = Prelu(f1rep + f2col) (ACT)  p' = Exp(u) (ACT)  p = p'*m01 (DVE)
  P4 : same ACT ops; the m01 multiply runs on GPSIMD
where B1 = exp(.8 f1_i) replicated, e1c = exp(f2_j), e2c = exp(.2 f2_j)
per-partition f32 scalars.  Dual-scalar tensor_scalar and the bf16
tensor ops give DVE 2-4x element rates; routes are chosen per pair to
jointly saturate DVE+ACT+GPSIMD (cost-model hill-climb).

Softmax division is deferred to the per-head-pair epilogue (1/S via DVE
reciprocal), fused with ELU via elu(v)+1 = relu(v) + exp(min(v,0)),
the +1 folded into a rank-1 correction matmul of the layer-2 projection.
"""

import numpy as np
import ml_dtypes

import concourse.bass as bass
import concourse.tile as tile
from concourse import mybir
from concourse.bass_utils import run_bass_kernel_spmd

F32 = mybir.dt.float32
F32R = mybir.dt.float32r
BF16 = mybir.dt.bfloat16

B, N, F, H, D, C = 4, 2048, 128, 8, 64, 32
I = N // 2          # query rows per core
JT = N // 128       # key tiles
IC = I // 128       # query-row 128-chunks per core
KT = (H * D) // 128 # hidden-dim 128-chunks
ALPHA = 0.2
BIG = 1e15          # mask bias; also dominates A*B in the factored path
N_CORES = 8
REPLICA_GROUPS = [[0, 1], [2, 3], [4, 5], [6, 7]]

ADD = mybir.AluOpType.add
MAX = mybir.AluOpType.max
MULT = mybir.AluOpType.mult
# NOTE: hardware "Lrelu" has a fixed 0.01 slope and ignores alpha;
# "Prelu" honors alpha (verified on HW) — it is the configurable leaky relu.
ACT_LRELU = mybir.ActivationFunctionType.Prelu
ACT_EXP = mybir.ActivationFunctionType.Exp
ACT_LN = mybir.ActivationFunctionType.Ln
ACT_COPY = mybir.ActivationFunctionType.Copy

N_HEADS_ALL = H + 1          # 8 layer-1 heads + the layer-2 output head
PAIRS_PER_HEAD = JT // 2     # 8
N_PAIRS = N_HEADS_ALL * PAIRS_PER_HEAD  # 72


def _split_multiwaits(nc):
    """Pinned walrus accepts only one sync-wait per instruction; Tile's exit
    drain (and occasionally others) carries several.  Hoist extras onto
    single-wait Drains on the same engine immediately before the owner."""
    n_fixed = 0
    for fn in nc.m.functions:
        for bb in fn.blocks:
            for name in [i.name for i in bb.instructions]:
                idx = [i.name for i in bb.instructions].index(name)
                inst = bb.instructions[idx]
                si = inst.sync_info
                if si is None or len(si.on_wait) <= 1:
                    continue
                waits = list(si.on_wait)
                for k, w in enumerate(waits[:-1]):
                    nd = mybir.InstDrain(
                        name=f"waitfix-{inst.name}-{k}", ins=[], outs=[])
                    nd.engine = inst.engine
                    nd.sync_info = mybir.SyncInfo(on_wait=[w], on_update=[])
                    nc.register_instruction(nd, overwrite=True)
                    bb.instructions.insert(idx + k, nd)
                inst.sync_info = mybir.SyncInfo(
                    on_wait=waits[-1:], on_update=list(si.on_update))
                n_fixed += 1
    return n_fixed


def _make_route(cfg):
    """Per-pair pipeline assignment:
      7 (DVE ts-dual + DVE m01-mult),   5 (DVE ts-dual + GPS m01-mult),
      3 (ACT prelu/exp + DVE m01-mult), 4 (ACT prelu/exp + GPS m01-mult).
    Counts spread across the 9 'heads' (8 L1 + L2)."""
    if "route" in cfg:                      # explicit per-pair override
        route = list(cfg["route"])
        assert len(route) == N_PAIRS
        return route
    # counts apply to the 64 layer-1 pairs; layer 2 is its own l2route
    n4 = int(cfg.get("p4", 0))
    n7 = int(cfg.get("p7", 17))
    n5 = int(cfg.get("p5", 25))
    n3 = H * PAIRS_PER_HEAD - n4 - n7 - n5
    assert n3 >= 0
    l2r = list(cfg.get("l2route", [5, 7, 7, 5, 7, 7, 7, 7]))
    assert len(l2r) == PAIRS_PER_HEAD
    # one largest-remainder interleave over all 64 L1 slots, chunked into
    # heads: every head gets a balanced, alternating mode mix
    want = {7: n7, 5: n5, 3: n3, 4: n4}
    nslots = H * PAIRS_PER_HEAD
    acc = {m: 0.0 for m in want}
    route = []
    for _ in range(nslots):
        for m in want:
            acc[m] += want[m]
        pick = max(want, key=lambda m: (acc[m], want[m]))
        acc[pick] -= nslots
        route.append(pick)
    route.extend(l2r)
    return route


def build_program(with_collective=True, cfg=None, repeat=1):
    cfg = dict(cfg or {})
    QB[0] = int(cfg.get("qbufs", 3))
    route = _make_route(cfg)

    nc = bass.Bass("TRN2", target_bir_lowering=False, debug=False,
                   enable_asserts=False, num_devices=N_CORES)

    xt_d = nc.dram_tensor("xt", [F, N], BF16, kind="ExternalInput")
    xtl_d = nc.dram_tensor("xtl", [F, I], BF16, kind="ExternalInput")
    mb_d = nc.dram_tensor("mb", [JT, 128, I], BF16, kind="ExternalInput")
    wext_d = nc.dram_tensor("wext", [H, F, D + 2], BF16, kind="ExternalInput")
    a1rep_d = nc.dram_tensor("a1rep", [H, F, 128], BF16, kind="ExternalInput")
    woext_d = nc.dram_tensor("woext", [KT, 128, C + 2], F32,
                             kind="ExternalInput")
    wcorr_d = nc.dram_tensor("wcorr", [1, C + 2], F32, kind="ExternalInput")
    ident_d = nc.dram_tensor("ident", [128, 128], F32, kind="ExternalInput")
    outp_d = nc.dram_tensor("outp", [I, C], F32, kind="ExternalOutput")

    with tile.TileContext(nc) as tc:
        if repeat > 1:
            def body(iv, unroll=None):
                _build_body(nc, tc, xt_d, xtl_d, mb_d, wext_d, a1rep_d,
                            woext_d, wcorr_d, ident_d, outp_d,
                            with_collective, route, cfg)
            with tc.For_i(0, repeat, 1) as iv:
                body(iv)
        else:
            _build_body(nc, tc, xt_d, xtl_d, mb_d, wext_d, a1rep_d,
                        woext_d, wcorr_d, ident_d, outp_d,
                        with_collective, route, cfg)
    _split_multiwaits(nc)
    return nc


QB = [3]


def _emit_pair(nc, work, workp, mode, subs, pair_args, mpair):
    """Emit one logit pair.  subs = [(jt, m01_ap, f2col_ap, e1col_ap,
    e2col_ap)]; pair_args = (f1rep, B1); mpair = [128,2,I] m01 view."""
    f1rep, B1 = pair_args
    if mode in (5, 7):
        q = work.tile([128, 2, I], BF16, tag="q1", bufs=QB[0])
        for k, (jt, m_ap, f2c, e1c, e2c) in enumerate(subs):
            # q = max(exp(.8 f1_i)*exp(f2_j), exp(.2 f2_j))  [one 4x TS op]
            nc.vector.tensor_scalar(q[:, k, :], B1[:], e1c, e2c, MULT, MAX)
        p = workp.tile([128, 2, I], BF16, tag="p")
        if mode == 5:
            for k in range(2):
                nc.gpsimd.tensor_mul(p[:, k, :], q[:, k, :], mpair[:, k, :])
        else:
            nc.vector.tensor_tensor(out=p[:], in0=q[:], in1=mpair, op=MULT)
        return p
    # ACT pipelines (3: DVE mask-mult, 4: GPS mask-mult)
    u = work.tile([128, 2, I], BF16, tag="u")
    for k, (jt, m_ap, f2c, e1c, e2c) in enumerate(subs):
        nc.scalar.activation(u[:, k, :], f1rep[:], ACT_LRELU,
                             bias=f2c, alpha=ALPHA)
    nc.scalar.activation(u[:], u[:], ACT_EXP)
    p = workp.tile([128, 2, I], BF16, tag="p")
    if mode == 4:
        for k in range(2):
            nc.gpsimd.tensor_mul(p[:, k, :], u[:, k, :], mpair[:, k, :])
    else:
        nc.vector.tensor_tensor(out=p[:], in0=u[:], in1=mpair, op=MULT)
    return p


def _copy_engine(nc, eng, out, in_):
    if eng == "act":
        nc.scalar.activation(out, in_, ACT_COPY)
    elif eng == "gps":
        nc.gpsimd.tensor_copy(out=out, in_=in_)
    else:
        nc.vector.tensor_copy(out=out, in_=in_)


def _build_body(nc, tc, xt_d, xtl_d, mb_d, wext_d, a1rep_d, woext_d,
                wcorr_d, ident_d, outp_d, with_collective, route, cfg):
    from contextlib import ExitStack
    ctx = ExitStack()
    f1rep_eng = cfg.get("f1rep_eng", "dve")
    rbc_eng = cfg.get("rbc_eng", "act")
    assert f1rep_eng != "gps" and rbc_eng != "gps"  # GPSIMD cannot read PSUM
    
    ph0_engs = cfg.get("ph0_engs", ("act", "dve"))
    ep_v_gps = False  # GPSIMD cannot read PSUM (hT)
    with ctx:
        singles = ctx.enter_context(tc.tile_pool(name="singles", bufs=1))
        psA = ctx.enter_context(tc.tile_pool(
            name="psA", bufs=int(cfg.get("psa", 2)), space="PSUM"))
        psB = ctx.enter_context(tc.tile_pool(
            name="psB", bufs=int(cfg.get("psb", 1)), space="PSUM"))
        psC = ctx.enter_context(tc.tile_pool(name="psC", bufs=2, space="PSUM"))
        dram = ctx.enter_context(tc.tile_pool(name="dram", bufs=1,
                                              space="DRAM"))

        # ---------------- persistent loads ----------------
        mb_s = singles.tile([128, JT, I], BF16)
        nc.sync.dma_start(out=mb_s[:, 0:2, :],
                          in_=mb_d.ap()[0:2].rearrange("jt p i -> p jt i"))
        xtl_s = singles.tile([F, I], BF16)
        nc.sync.dma_start(out=xtl_s[:], in_=xtl_d.ap())
        a1rep_s = singles.tile([F, H, 128], BF16)
        nc.sync.dma_start(out=a1rep_s[:],
                          in_=a1rep_d.ap().rearrange("h f e -> f h e"))
        wcorr_s = singles.tile([1, C + 2], F32)
        nc.sync.dma_start(out=wcorr_s[:], in_=wcorr_d.ap())
        ident_s = singles.tile([128, 128], F32)
        nc.sync.dma_start(out=ident_s[:], in_=ident_d.ap())
        woext_raw = singles.tile([128, KT, C + 2], F32)
        nc.sync.dma_start(out=woext_raw[:],
                          in_=woext_d.ap().rearrange("k f e -> f k e"))
        woext_s = singles.tile([128, KT, C + 2], F32R)
        nc.vector.tensor_copy(out=woext_s[:], in_=woext_raw[:])

        ones_s = singles.tile([1, 128], BF16)
        nc.gpsimd.memset(ones_s[:], 1.0)
        onesf_s = singles.tile([1, 128], F32)
        nc.gpsimd.memset(onesf_s[:], 1.0)

        whbuf = singles.tile([128, H, JT, D + 1], BF16)
        nc.gpsimd.memset(whbuf[:, :, :, D:D + 1], 1.0)
        fcol = singles.tile([128, H, JT, 1], F32)
        fexp1 = singles.tile([128, H, JT, 1], F32)
        fexp2 = singles.tile([128, H, JT, 1], F32)
        hcatT = singles.tile([128, KT, I], F32R)

        xt_s = singles.tile([F, N], BF16)
        nc.sync.dma_start(out=xt_s[:], in_=xt_d.ap())
        wext_s = singles.tile([F, H, D + 2], BF16)
        nc.sync.dma_start(out=wext_s[:],
                          in_=wext_d.ap().rearrange("h f e -> f h e"))
        for j0 in (2, 6, 10):
            j1 = j0 + 4 if j0 < 10 else JT
            nc.sync.dma_start(
                out=mb_s[:, j0:j1, :],
                in_=mb_d.ap()[j0:j1].rearrange("jt p i -> p jt i"))

        work = ctx.enter_context(
            tc.tile_pool(name="work", bufs=int(cfg.get("wbufs", 4))))
        workp = ctx.enter_context(
            tc.tile_pool(name="workp", bufs=int(cfg.get("pbufs", 4))))
        ep1 = ctx.enter_context(tc.tile_pool(name="ep1", bufs=1))
        ep2 = ctx.enter_context(tc.tile_pool(name="ep2", bufs=2))
        epL2 = ctx.enter_context(tc.tile_pool(name="epL2", bufs=1))

        def emit_phase0_head(h):
            # Wh tiles + f columns for head h (emitted per-head so the
            # copies overlap the previous head's logit work)
            for jg in range(JT // 4):
                whp = psA.tile([128, 4, D + 2], F32, tag="ph")
                for k in range(4):
                    jt = jg * 4 + k
                    nc.tensor.matmul(whp[:, k, :],
                                     lhsT=xt_s[:, jt * 128:(jt + 1) * 128],
                                     rhs=wext_s[:, h, :])
                dst = whbuf[:, h, jg * 4:(jg + 1) * 4, 0:D]
                _copy_engine(nc, ph0_engs[jg % len(ph0_engs)],
                             dst, whp[:, :, 0:D])
                nc.vector.tensor_copy(
                    out=fcol[:, h, jg * 4:(jg + 1) * 4, :],
                    in_=whp[:, :, D + 1:D + 2])
            nc.scalar.activation(fexp1[:, h], fcol[:, h], ACT_EXP)
            nc.scalar.activation(fexp2[:, h], fcol[:, h], ACT_EXP,
                                 scale=ALPHA)

        ep_state = {}

        def emit_half_ep(hT, h, sliced=False):
            # per-head half-epilogue: rinv = 1/S via DVE reciprocal, PE
            # partition broadcast, v-half = hT*rinv.  The odd half finishes:
            # hcat = elu(v)+1 = relu(v)+exp(min(v,0)).  `sliced` pipelines
            # the chain in 512-column slices (used for the final head-pair,
            # where this chain gates the whole layer-2 tail).
            rinv = ep1.tile([1, I], F32, tag=f"ri{h % 2}", bufs=1)
            if h % 2 == 0:
                v = ep1.tile([128, I], BF16, tag="v", bufs=2)
                ep_state["v"] = v
                half = slice(0, D)
            else:
                v = ep_state["v"]
                half = slice(D, 128)
            rbp = psB.tile([128, I], F32, tag="rep")
            # HW: a DVE op may read only ONE input from PSUM, so the
            # broadcast row block is staged through SBUF (rbc)
            rbc = ep1.tile([D, I], F32, tag=f"rb{h % 2}", bufs=1)
            t = None
            if h % 2 == 1:
                t = ep1.tile([128, I], BF16, tag="t", bufs=2)
            for hf_ in range(I // 512):
                sl_ = slice(hf_ * 512, (hf_ + 1) * 512)
                nc.vector.reciprocal(rinv[0:1, sl_], hT[D:D + 1, sl_])
                nc.tensor.matmul(rbp[0:D, sl_], lhsT=onesf_s[0:1, 0:D],
                                 rhs=rinv[0:1, sl_])
                if sliced:
                    _copy_engine(nc, rbc_eng, rbc[:, sl_], rbp[0:D, sl_])
                    nc.vector.tensor_tensor(out=v[half, sl_],
                                            in0=hT[0:D, sl_],
                                            in1=rbc[:, sl_], op=MULT)
                    if h % 2 == 1:
                        nc.vector.tensor_scalar_min(t[:, sl_], v[:, sl_], 0.0)
                        nc.scalar.activation(t[:, sl_], t[:, sl_], ACT_EXP)
                        nc.vector.scalar_tensor_tensor(
                            out=hcatT[:, h // 2, sl_], in0=v[:, sl_],
                            scalar=0.0, in1=t[:, sl_], op0=MAX, op1=ADD)
            if not sliced:
                _copy_engine(nc, rbc_eng, rbc[:], rbp[0:D, :])
                nc.vector.tensor_tensor(out=v[half, :], in0=hT[0:D, :],
                                        in1=rbc[:], op=MULT)
                if h % 2 == 1:
                    nc.vector.tensor_scalar_min(t[:], v[:], 0.0)
                    nc.scalar.activation(t[:], t[:], ACT_EXP)
                    nc.vector.scalar_tensor_tensor(
                        out=hcatT[:, h // 2, :], in0=v[:], scalar=0.0,
                        in1=t[:], op0=MAX, op1=ADD)

        def emit_head_prep(h):
            emit_phase0_head(h)
            head_modes = route[h * PAIRS_PER_HEAD:(h + 1) * PAIRS_PER_HEAD]
            need_f1rep = any(m in (3, 4) for m in head_modes)
            need_B = any(m in (5, 7) for m in head_modes)
            f1p = psB.tile([128, I], F32, tag="rep")
            for hf in range(I // 512):
                sl = slice(hf * 512, (hf + 1) * 512)
                nc.tensor.matmul(f1p[:, sl], lhsT=a1rep_s[:, h, :],
                                 rhs=xtl_s[:, sl])
            f1rep_s = B1_s = None
            if need_f1rep:
                f1rep_s = ep2.tile([128, I], BF16, tag="f1rep")
                _copy_engine(nc, f1rep_eng, f1rep_s[:], f1p[:])
            if need_B:
                B1_s = ep2.tile([128, I], BF16, tag="B1")
                nc.scalar.activation(B1_s[:], f1p[:], ACT_EXP, scale=1.0 - ALPHA)
            return f1rep_s, B1_s

        # ---------------- layer 1 ----------------
        pending_ep = None   # deferred half-epilogue (software pipelining)
        preps = emit_head_prep(0)
        for h in range(H):
            head_modes = route[h * PAIRS_PER_HEAD:(h + 1) * PAIRS_PER_HEAD]
            cur = preps
            hT = psC.tile([D + 1, I], F32, tag="acc")
            for jp in range(PAIRS_PER_HEAD):
                if jp == int(cfg.get("ep_defer", 2)) and pending_ep is not None:
                    pending_ep()
                    pending_ep = None
                if jp == int(cfg.get("prep_at", 4)) and h + 1 < H:
                    preps = emit_head_prep(h + 1)
                mode = head_modes[jp]
                subs = []
                for k in range(2):
                    jt = jp * 2 + k
                    subs.append((jt, mb_s[:, jt, :],
                                 fcol[:, h, jt, :], fexp1[:, h, jt, :],
                                 fexp2[:, h, jt, :]))
                p = _emit_pair(nc, work, workp, mode, subs, cur,
                               mb_s[:, jp * 2:jp * 2 + 2, :])
                for k in range(2):
                    jt = jp * 2 + k
                    for hf in range(I // 512):
                        sl = slice(hf * 512, (hf + 1) * 512)
                        nc.tensor.matmul(hT[:, sl],
                                         lhsT=whbuf[:, h, jt, :],
                                         rhs=p[:, k, sl],
                                         start=(jt == 0), stop=(jt == JT - 1))

            pending_ep = (lambda t_=hT, h_=h, s_=(h == H - 1):
                          emit_half_ep(t_, h_, sliced=s_))
        if pending_ep is not None:
            pending_ep()
            pending_ep = None

        # ---------------- layer 2 projection + exchange -------------------
        # HOST permutes the key order per core to [my I queries; partner's I
        # queries], so key tiles jt 0..7 are LOCAL (read straight from
        # wh2loc, no collective round-trip) and only tiles 8..15 need the
        # partner's projection.  The exchange is an AllReduce(add) of the
        # local projection; partner = sum - mine (exact to f32 rounding).
        wh2loc = singles.tile([128, IC, C + 2], F32)
        gin = dram.tile([I, C + 2], F32)
        g1rowp = psB.tile([128, I], F32, tag="rep")
        for ic in range(IC):
            w2p = psA.tile([128, 4, D + 2], F32, tag="ph")
            for kt in range(KT):
                nc.tensor.matmul(
                    w2p[:, 0, 0:C + 2],
                    lhsT=hcatT[:, kt, ic * 128:(ic + 1) * 128],
                    rhs=woext_s[:, kt, :],
                    start=(kt == 0), stop=False)
            nc.tensor.matmul(w2p[:, 0, 0:C + 2], lhsT=onesf_s[0:1, :],
                             rhs=wcorr_s[:], start=False, stop=True)
            nc.vector.tensor_copy(out=wh2loc[:, ic, :], in_=w2p[:, 0, 0:C + 2])
            nc.tensor.transpose(g1rowp[0:1, ic * 128:(ic + 1) * 128],
                                in_=wh2loc[:, ic, 0:1], identity=ident_s[:])
        for hf in range(2):
            nc.sync.dma_start(
                out=gin[hf * 512:(hf + 1) * 512, :].rearrange(
                    "(ic p) c -> p ic c", p=128),
                in_=wh2loc[:, hf * 4:(hf + 1) * 4, :])

        gsum = dram.tile([I, C + 2], F32)
        if with_collective:
            nc.gpsimd.collective_compute(
                "AllReduce", mybir.AluOpType.add,
                replica_groups=REPLICA_GROUPS,
                ins=[gin.opt()], outs=[gsum.opt()])
        else:  # timing-model variant: fake the exchange with a local copy
            nc.sync.dma_start(out=gsum[:], in_=gin[:])

        # g1 row (local queries) -> replicated [128, I]
        g1row_s = epL2.tile([1, I], BF16, tag="g1row")
        nc.scalar.activation(g1row_s[:], g1rowp[0:1, :], ACT_COPY)
        g1rp = psB.tile([128, I], F32, tag="rep")
        for hf in range(I // 512):
            sl = slice(hf * 512, (hf + 1) * 512)
            nc.tensor.matmul(g1rp[:, sl], lhsT=ones_s[0:1, :],
                             rhs=g1row_s[0:1, sl])
        l2_modes = route[H * PAIRS_PER_HEAD:]
        g1rep_s = B1L2 = None
        if any(m in (3, 4) for m in l2_modes):
            g1rep_s = singles.tile([128, I], BF16)
            nc.vector.tensor_copy(out=g1rep_s[:], in_=g1rp[:])
        if any(m in (5, 7) for m in l2_modes):
            B1L2 = singles.tile([128, I], BF16)
            nc.scalar.activation(B1L2[:], g1rp[:], ACT_EXP, scale=1.0 - ALPHA)

        # key-side rows: [g1, g2, Wh2(32)] f32, bf16 for the PV lhsT.
        # Local tiles (jt 0..7) come straight from wh2loc; remote tiles
        # (jt 8..15) from the AllReduce sum minus the local projection.
        JH = JT // 2
        wh2gr = singles.tile([128, JT, C + 3], BF16)
        nc.gpsimd.memset(wh2gr[:, :, C + 2:C + 3], 1.0)
        its1 = singles.tile([128, JT, 1], F32)
        its2 = singles.tile([128, JT, 1], F32)
        for jg in range(JH // 4):
            s4 = slice(jg * 4, (jg + 1) * 4)
            nc.gpsimd.tensor_copy(out=wh2gr[:, s4, 0:C + 2],
                                  in_=wh2loc[:, s4, :])
            nc.scalar.activation(its1[:, s4, :], wh2loc[:, s4, 1:2], ACT_EXP)
            nc.scalar.activation(its2[:, s4, :], wh2loc[:, s4, 1:2], ACT_EXP,
                                 scale=ALPHA)
        wh2sum = singles.tile([128, JH, C + 2], F32)
        nc.sync.dma_start(
            out=wh2sum[:],
            in_=gsum.rearrange("(jt p) c -> p jt c", p=128))
        wh2rem = singles.tile([128, JH, C + 2], F32)
        nc.vector.tensor_tensor(out=wh2rem[:], in0=wh2sum[:],
                                in1=wh2loc[:], op=mybir.AluOpType.subtract)
        for jg in range(JH // 4):
            s4 = slice(jg * 4, (jg + 1) * 4)
            s4r = slice(JH + jg * 4, JH + (jg + 1) * 4)
            nc.gpsimd.tensor_copy(out=wh2gr[:, s4r, 0:C + 2],
                                  in_=wh2rem[:, s4, :])
            nc.scalar.activation(its1[:, s4r, :], wh2rem[:, s4, 1:2], ACT_EXP)
            nc.scalar.activation(its2[:, s4r, :], wh2rem[:, s4, 1:2], ACT_EXP,
                                 scale=ALPHA)

        # ---------------- layer 2 attention ----------------
        o2T = psC.tile([D + 1, I], F32, tag="acc")
        for jp in range(PAIRS_PER_HEAD):
            mode = l2_modes[jp]
            subs = []
            for k in range(2):
                jt = jp * 2 + k
                f2c = (wh2loc[:, jt, 1:2] if jt < JH
                       else wh2rem[:, jt - JH, 1:2])
                subs.append((jt, mb_s[:, jt, :],
                             f2c, its1[:, jt, :],
                             its2[:, jt, :]))
            p = _emit_pair(nc, work, workp, mode, subs,
                           (g1rep_s, B1L2), mb_s[:, jp * 2:jp * 2 + 2, :])
            for k in range(2):
                jt = jp * 2 + k
                for hf in range(I // 512):
                    sl = slice(hf * 512, (hf + 1) * 512)
                    nc.tensor.matmul(o2T[0:C + 1, sl],
                                     lhsT=wh2gr[:, jt, 2:C + 3],
                                     rhs=p[:, k, sl],
                                     start=(jt == 0), stop=(jt == JT - 1))

        # ---------------- finalize (transposed: per-query reciprocal) -----
        # per 512-slice: rinv2 (DVE recip) -> PE row-broadcast -> oT =
        # o2T * rbc2 (PSUM x SBUF) -> per-128 transposes -> out DMA halves
        rinv2 = epL2.tile([1, I], F32, tag="ri2")
        oT_s = epL2.tile([C, I], F32, tag="oT")
        ofs_all = epL2.tile([128, IC, C], F32, tag="ofs_all")
        rbc2p = psB.tile([128, I], F32, tag="rep")
        rbc2_s = epL2.tile([C, I], F32, tag="rbc2")
        for hf in range(I // 512):
            sl = slice(hf * 512, (hf + 1) * 512)
            nc.vector.reciprocal(rinv2[0:1, sl], o2T[C:C + 1, sl])
            nc.tensor.matmul(rbc2p[0:C, sl], lhsT=onesf_s[0:1, 0:C],
                             rhs=rinv2[0:1, sl])
            _copy_engine(nc, rbc_eng, rbc2_s[:, sl], rbc2p[0:C, sl])
            nc.vector.tensor_tensor(out=oT_s[:, sl], in0=o2T[0:C, sl],
                                    in1=rbc2_s[:, sl], op=MULT)
            for k in range(hf * 4, hf * 4 + 4):
                ofp = psA.tile([128, 4, D + 2], F32, tag="ph")
                nc.tensor.transpose(ofp[:, 0, 0:C],
                                    in_=oT_s[:, k * 128:(k + 1) * 128],
                                    identity=ident_s[0:C, 0:C])
                nc.vector.tensor_copy(out=ofs_all[:, k, :],
                                      in_=ofp[:, 0, 0:C])
            nc.sync.dma_start(
                out=outp_d.ap()[hf * 512:(hf + 1) * 512, :].rearrange(
                    "(k p) c -> p k c", p=128),
                in_=ofs_all[:, hf * 4:hf * 4 + 4, :])


# --------------------------------------------------------------------------
# host side
# --------------------------------------------------------------------------

def shard_inputs(x, adj, W, a1, a2, Wo, ao1, ao2):
    x = np.asarray(x, np.float32)
    adj = np.asarray(adj)
    W = np.asarray(W, np.float32)
    a1 = np.asarray(a1, np.float32)
    a2 = np.asarray(a2, np.float32)
    Wo = np.asarray(Wo, np.float32)
    ao1 = np.asarray(ao1, np.float32)
    ao2 = np.asarray(ao2, np.float32)
    BF = ml_dtypes.bfloat16

    wvec1 = np.einsum("hfd,hd->hf", W, a1)          # [H, F]
    wvec2 = np.einsum("hfd,hd->hf", W, a2)
    wext = np.concatenate([W, wvec1[:, :, None], wvec2[:, :, None]],
                          axis=2).astype(BF)
    a1rep = np.repeat(wvec1[:, :, None], 128, axis=2).astype(BF)
    wo1 = Wo @ ao1                                   # [512]
    wo2 = Wo @ ao2
    woflat = np.concatenate([wo1[:, None], wo2[:, None], Wo], 1)  # [512, 34]
    woext = woflat.reshape(KT, 128, C + 2).astype(np.float32)
    wcorr = (-woflat.sum(0))[None, :].astype(np.float32)
    ident = np.eye(128, dtype=np.float32)

    in_maps = []
    for c in range(N_CORES):
        b, half = c // 2, c % 2
        i0 = half * I
        # Per-core key order [my I queries; partner's I queries]: key tiles
        # jt 0..7 are then LOCAL in layer 2 (see kernel L2 exchange).
        # Attention is key-order invariant as long as xt and the mask rows
        # are permuted consistently.
        perm = np.r_[i0:i0 + I, (I - i0):(I - i0) + I]
        xt = np.ascontiguousarray(x[b].T[:, perm]).astype(BF)   # [F, N]
        xtl = np.ascontiguousarray(xt[:, 0:I])
        adjt = adj[b, i0:i0 + I, :].T[perm, :]       # [N, I] = (j, i)
        mb = np.where(adjt > 0, np.float32(1.0), np.float32(0.0))
        mb = np.ascontiguousarray(mb.reshape(JT, 128, I)).astype(BF)
        in_maps.append({
            "xt": xt, "xtl": xtl, "mb": mb, "wext": wext,
            "a1rep": a1rep, "woext": woext, "wcorr": wcorr, "ident": ident,
        })
    return in_maps


# Engine routing: multiplicative-mask pipelines balanced across
# DVE (P7) / DVE+GPS (P5) / ACT+DVE (P3) / ACT+GPS (P4) by cost-model
# (TimelineSim) hill-climb.
DEFAULT_CFG = {"ep_defer": 2, "f1rep_eng": "act", "rbc_eng": "act",
               "prep_at": 3, "p4": 0, "p7": 20, "p5": 24,
               "pbufs": 8, "wbufs": 6, "qbufs": 4}

_CACHE = {}


def _program():
    if "nc" not in _CACHE:
        _CACHE["nc"] = build_program(with_collective=True, cfg=DEFAULT_CFG)
    return _CACHE["nc"]


def kernel(**inputs):
    nc = _program()
    in_maps = shard_inputs(**inputs)
    res = run_bass_kernel_spmd(nc, in_maps, list(range(N_CORES)))
    _CACHE["last_results"] = res
    out = np.empty((B, N, C), np.float32)
    for c in range(N_CORES):
        b, half = c // 2, c % 2
        out[b, half * I:(half + 1) * I, :] = res.results[c]["outp"]
    return out



# revision 40
# speedup vs baseline: 1.4202x; 1.0419x over previous
"""GAT forward (2-layer graph attention, B=4 N=2048 F=128 H=8 D=64 C=32)
as a Bass/Tile SPMD kernel on 8 Trainium2 NeuronCores.

Sharding: core c -> (batch b=c//2, query-row half c%2).  Each core computes
attention for its 1024 query rows over all 2048 keys for all 8 heads
(layer 1) and for the output head (layer 2).  The only cross-core exchange
is a 2-rank AllGather of the layer-2 projections [g1|g2|Wh2] ([1024,34] f32)
within each (2b, 2b+1) pair.

Layout: attention logits are built TRANSPOSED, e^T[j (keys) = partitions,
i (queries) = free], so the PV matmul needs no operand transposes
(lhsT = Wh[j,d] stationary, rhs = p[j,i] moving, out = h^T[d,i]) and
softmax row sums come from a ones-column appended to Wh (PSUM row D).

The N^2 elementwise work over 144 [128,1024] logit tiles is the bottleneck.
Key identity: after a softmax-invariant per-query rescale by exp(-.2 f1_i),
  exp(prelu(f1_i + f2_j)) = max(exp(.8 f1_i)*exp(f2_j), exp(.2 f2_j))
and the adjacency mask is applied MULTIPLICATIVELY (m01 in {0,1}), so a
pair of logit tiles [128,2,1024] needs only:
  P7 : q = ts(B1, e1c, e2c, mult, max)  per sub   (DVE tensor-scalar 4x!)
       p = q * m01                      per pair  (DVE tensor-tensor 2x)
  P5 : same q on DVE; p = q * m01 on GPSIMD (tensor_mult)
  P3 : u = Prelu(f1rep + f2col) (ACT)  p' = Exp(u) (ACT)  p = p'*m01 (DVE)
  P4 : same ACT ops; the m01 multiply runs on GPSIMD
where B1 = exp(.8 f1_i) replicated, e1c = exp(f2_j), e2c = exp(.2 f2_j)
per-partition f32 scalars.  Dual-scalar tensor_scalar and the bf16
tensor ops give DVE 2-4x element rates; routes are chosen per pair to
jointly saturate DVE+ACT+GPSIMD (cost-model hill-climb).

Softmax division is deferred to the per-head-pair epilogue (1/S via DVE
reciprocal), fused with ELU via elu(v)+1 = relu(v) + exp(min(v,0)),
the +1 folded into a rank-1 correction matmul of the layer-2 projection.
"""

import numpy as np
import ml_dtypes

import concourse.bass as bass
import concourse.tile as tile
from concourse import mybir
from concourse.bass_utils import run_bass_kernel_spmd

F32 = mybir.dt.float32
F32R = mybir.dt.float32r
BF16 = mybir.dt.bfloat16

B, N, F, H, D, C = 4, 2048, 128, 8, 64, 32
I = N // 2          # query rows per core
JT = N // 128       # key tiles
IC = I // 128       # query-row 128-chunks per core
KT = (H * D) // 128 # hidden-dim 128-chunks
ALPHA = 0.2
BIG = 1e15          # mask bias; also dominates A*B in the factored path
N_CORES = 8
REPLICA_GROUPS = [[0, 1], [2, 3], [4, 5], [6, 7]]

ADD = mybir.AluOpType.add
MAX = mybir.AluOpType.max
MULT = mybir.AluOpType.mult
# NOTE: hardware "Lrelu" has a fixed 0.01 slope and ignores alpha;
# "Prelu" honors alpha (verified on HW) — it is the configurable leaky relu.
ACT_LRELU = mybir.ActivationFunctionType.Prelu
ACT_EXP = mybir.ActivationFunctionType.Exp
ACT_LN = mybir.ActivationFunctionType.Ln
ACT_COPY = mybir.ActivationFunctionType.Copy
ACT_RECIP = mybir.ActivationFunctionType.Reciprocal

N_HEADS_ALL = H + 1          # 8 layer-1 heads + the layer-2 output head
PAIRS_PER_HEAD = JT // 2     # 8
N_PAIRS = N_HEADS_ALL * PAIRS_PER_HEAD  # 72


def _split_multiwaits(nc):
    """Pinned walrus accepts only one sync-wait per instruction; Tile's exit
    drain (and occasionally others) carries several.  Hoist extras onto
    single-wait Drains on the same engine immediately before the owner."""
    n_fixed = 0
    for fn in nc.m.functions:
        for bb in fn.blocks:
            for name in [i.name for i in bb.instructions]:
                idx = [i.name for i in bb.instructions].index(name)
                inst = bb.instructions[idx]
                si = inst.sync_info
                if si is None or len(si.on_wait) <= 1:
                    continue
                waits = list(si.on_wait)
                for k, w in enumerate(waits[:-1]):
                    nd = mybir.InstDrain(
                        name=f"waitfix-{inst.name}-{k}", ins=[], outs=[])
                    nd.engine = inst.engine
                    nd.sync_info = mybir.SyncInfo(on_wait=[w], on_update=[])
                    nc.register_instruction(nd, overwrite=True)
                    bb.instructions.insert(idx + k, nd)
                inst.sync_info = mybir.SyncInfo(
                    on_wait=waits[-1:], on_update=list(si.on_update))
                n_fixed += 1
    return n_fixed


def _make_route(cfg):
    """Per-pair pipeline assignment:
      7 (DVE ts-dual + DVE m01-mult),   5 (DVE ts-dual + GPS m01-mult),
      3 (ACT prelu/exp + DVE m01-mult), 4 (ACT prelu/exp + GPS m01-mult).
    Counts spread across the 9 'heads' (8 L1 + L2)."""
    if "route" in cfg:                      # explicit per-pair override
        route = list(cfg["route"])
        assert len(route) == N_PAIRS
        return route
    # counts apply to the 64 layer-1 pairs; layer 2 is its own l2route
    n4 = int(cfg.get("p4", 0))
    n7 = int(cfg.get("p7", 17))
    n5 = int(cfg.get("p5", 25))
    n3 = H * PAIRS_PER_HEAD - n4 - n7 - n5
    assert n3 >= 0
    l2r = list(cfg.get("l2route", [5, 7, 7, 5, 7, 7, 7, 7]))
    assert len(l2r) == PAIRS_PER_HEAD
    # one largest-remainder interleave over all 64 L1 slots, chunked into
    # heads: every head gets a balanced, alternating mode mix
    want = {7: n7, 5: n5, 3: n3, 4: n4}
    nslots = H * PAIRS_PER_HEAD
    acc = {m: 0.0 for m in want}
    route = []
    for _ in range(nslots):
        for m in want:
            acc[m] += want[m]
        pick = max(want, key=lambda m: (acc[m], want[m]))
        acc[pick] -= nslots
        route.append(pick)
    route.extend(l2r)
    return route


def build_program(with_collective=True, cfg=None, repeat=1):
    cfg = dict(cfg or {})
    QB[0] = int(cfg.get("qbufs", 3))
    route = _make_route(cfg)

    nc = bass.Bass("TRN2", target_bir_lowering=False, debug=False,
                   enable_asserts=False, num_devices=N_CORES)

    xt_d = nc.dram_tensor("xt", [F, N], BF16, kind="ExternalInput")
    xtl_d = nc.dram_tensor("xtl", [F, I], BF16, kind="ExternalInput")
    mb_d = nc.dram_tensor("mb", [JT, 128, I], BF16, kind="ExternalInput")
    wext_d = nc.dram_tensor("wext", [H, F, D + 2], BF16, kind="ExternalInput")
    a1rep_d = nc.dram_tensor("a1rep", [H, F, 128], BF16, kind="ExternalInput")
    woext_d = nc.dram_tensor("woext", [KT, 128, C + 2], BF16,
                             kind="ExternalInput")
    wcorr_d = nc.dram_tensor("wcorr", [1, C + 2], F32, kind="ExternalInput")
    ident_d = nc.dram_tensor("ident", [128, 128], F32, kind="ExternalInput")
    outp_d = nc.dram_tensor("outp", [I, C], F32, kind="ExternalOutput")

    with tile.TileContext(nc) as tc:
        if repeat > 1:
            def body(iv, unroll=None):
                _build_body(nc, tc, xt_d, xtl_d, mb_d, wext_d, a1rep_d,
                            woext_d, wcorr_d, ident_d, outp_d,
                            with_collective, route, cfg)
            with tc.For_i(0, repeat, 1) as iv:
                body(iv)
        else:
            _build_body(nc, tc, xt_d, xtl_d, mb_d, wext_d, a1rep_d,
                        woext_d, wcorr_d, ident_d, outp_d,
                        with_collective, route, cfg)
    _split_multiwaits(nc)
    return nc


QB = [3]


def _emit_pair(nc, work, workp, mode, subs, pair_args, mpair):
    """Emit one logit pair.  subs = [(jt, m01_ap, f2col_ap, e1col_ap,
    e2col_ap)]; pair_args = (f1rep, B1); mpair = [128,2,I] m01 view."""
    f1rep, B1 = pair_args
    if mode in (5, 7):
        q = work.tile([128, 2, I], BF16, tag="q1", bufs=QB[0])
        for k, (jt, m_ap, f2c, e1c, e2c) in enumerate(subs):
            # q = max(exp(.8 f1_i)*exp(f2_j), exp(.2 f2_j))  [one 4x TS op]
            nc.vector.tensor_scalar(q[:, k, :], B1[:], e1c, e2c, MULT, MAX)
        p = workp.tile([128, 2, I], BF16, tag="p")
        if mode == 5:
            for k in range(2):
                nc.gpsimd.tensor_mul(p[:, k, :], q[:, k, :], mpair[:, k, :])
        else:
            nc.vector.tensor_tensor(out=p[:], in0=q[:], in1=mpair, op=MULT)
        return p
    # ACT pipelines (3: DVE mask-mult, 4: GPS mask-mult)
    u = work.tile([128, 2, I], BF16, tag="u")
    for k, (jt, m_ap, f2c, e1c, e2c) in enumerate(subs):
        nc.scalar.activation(u[:, k, :], f1rep[:], ACT_LRELU,
                             bias=f2c, alpha=ALPHA)
    nc.scalar.activation(u[:], u[:], ACT_EXP)
    p = workp.tile([128, 2, I], BF16, tag="p")
    if mode == 4:
        for k in range(2):
            nc.gpsimd.tensor_mul(p[:, k, :], u[:, k, :], mpair[:, k, :])
    else:
        nc.vector.tensor_tensor(out=p[:], in0=u[:], in1=mpair, op=MULT)
    return p


def _copy_engine(nc, eng, out, in_):
    if eng == "act":
        nc.scalar.activation(out, in_, ACT_COPY)
    elif eng == "gps":
        nc.gpsimd.tensor_copy(out=out, in_=in_)
    else:
        nc.vector.tensor_copy(out=out, in_=in_)


def _build_body(nc, tc, xt_d, xtl_d, mb_d, wext_d, a1rep_d, woext_d,
                wcorr_d, ident_d, outp_d, with_collective, route, cfg):
    from contextlib import ExitStack
    ctx = ExitStack()
    f1rep_eng = cfg.get("f1rep_eng", "dve")
    rbc_eng = cfg.get("rbc_eng", "act")
    fcol_eng = cfg.get("fcol_eng", "dve")
    rinv_eng = cfg.get("rinv_eng", "dve")
    stt_eng = cfg.get("stt_eng", "dve")
    assert f1rep_eng != "gps" and rbc_eng != "gps"  # GPSIMD cannot read PSUM
    
    ph0_engs = cfg.get("ph0_engs", ("act", "dve"))
    ep_v_gps = False  # GPSIMD cannot read PSUM (hT)
    with ctx:
        singles = ctx.enter_context(tc.tile_pool(name="singles", bufs=1))
        psA = ctx.enter_context(tc.tile_pool(
            name="psA", bufs=int(cfg.get("psa", 2)), space="PSUM"))
        psB = ctx.enter_context(tc.tile_pool(
            name="psB", bufs=int(cfg.get("psb", 1)), space="PSUM"))
        psC = ctx.enter_context(tc.tile_pool(name="psC", bufs=2, space="PSUM"))
        dram = ctx.enter_context(tc.tile_pool(name="dram", bufs=1,
                                              space="DRAM"))

        # ---------------- persistent loads ----------------
        mb_s = singles.tile([128, JT, I], BF16)
        nc.sync.dma_start(out=mb_s[:, 0:2, :],
                          in_=mb_d.ap()[0:2].rearrange("jt p i -> p jt i"))
        xtl_s = singles.tile([F, I], BF16)
        nc.sync.dma_start(out=xtl_s[:], in_=xtl_d.ap())
        a1rep_s = singles.tile([F, H, 128], BF16)
        nc.sync.dma_start(out=a1rep_s[:],
                          in_=a1rep_d.ap().rearrange("h f e -> f h e"))
        wcorr_s = singles.tile([1, C + 2], F32)
        nc.sync.dma_start(out=wcorr_s[:], in_=wcorr_d.ap())
        ident_s = singles.tile([128, 128], F32)
        nc.sync.dma_start(out=ident_s[:], in_=ident_d.ap())
        woext_s = singles.tile([128, KT, C + 2], BF16)
        nc.sync.dma_start(out=woext_s[:],
                          in_=woext_d.ap().rearrange("k f e -> f k e"))

        ones_s = singles.tile([1, 128], BF16)
        nc.gpsimd.memset(ones_s[:], 1.0)
        onesf_s = singles.tile([1, 128], F32)
        nc.gpsimd.memset(onesf_s[:], 1.0)

        whbuf = singles.tile([128, H, JT, D + 1], BF16)
        nc.gpsimd.memset(whbuf[:, :, :, D:D + 1], 1.0)
        fcol = singles.tile([128, H, JT, 1], F32)
        fexp1 = singles.tile([128, H, JT, 1], F32)
        fexp2 = singles.tile([128, H, JT, 1], F32)
        hcatT = singles.tile([128, KT, I], BF16)

        xt_s = singles.tile([F, N], BF16)
        nc.sync.dma_start(out=xt_s[:], in_=xt_d.ap())
        wext_s = singles.tile([F, H, D + 2], BF16)
        nc.sync.dma_start(out=wext_s[:],
                          in_=wext_d.ap().rearrange("h f e -> f h e"))
        for j0 in (2, 6, 10):
            j1 = j0 + 4 if j0 < 10 else JT
            nc.sync.dma_start(
                out=mb_s[:, j0:j1, :],
                in_=mb_d.ap()[j0:j1].rearrange("jt p i -> p jt i"))

        work = ctx.enter_context(
            tc.tile_pool(name="work", bufs=int(cfg.get("wbufs", 4))))
        workp = ctx.enter_context(
            tc.tile_pool(name="workp", bufs=int(cfg.get("pbufs", 4))))
        ep1 = ctx.enter_context(tc.tile_pool(name="ep1", bufs=1))
        ep2 = ctx.enter_context(tc.tile_pool(name="ep2", bufs=2))
        epL2 = ctx.enter_context(tc.tile_pool(name="epL2", bufs=1))

        def emit_phase0_head(h):
            # Wh tiles + f columns for head h (emitted per-head so the
            # copies overlap the previous head's logit work)
            for jg in range(JT // 4):
                whp = psA.tile([128, 4, D + 2], F32, tag="ph")
                for k in range(4):
                    jt = jg * 4 + k
                    nc.tensor.matmul(whp[:, k, :],
                                     lhsT=xt_s[:, jt * 128:(jt + 1) * 128],
                                     rhs=wext_s[:, h, :])
                dst = whbuf[:, h, jg * 4:(jg + 1) * 4, 0:D]
                _copy_engine(nc, ph0_engs[jg % len(ph0_engs)],
                             dst, whp[:, :, 0:D])
                _copy_engine(nc, fcol_eng,
                             fcol[:, h, jg * 4:(jg + 1) * 4, :],
                             whp[:, :, D + 1:D + 2])
            nc.scalar.activation(fexp1[:, h], fcol[:, h], ACT_EXP)
            nc.scalar.activation(fexp2[:, h], fcol[:, h], ACT_EXP,
                                 scale=ALPHA)

        ep_state = {}

        def _stt_hcat(out_ap, v_ap, t_ap):
            eng = nc.gpsimd if stt_eng == "gps" else nc.vector
            eng.scalar_tensor_tensor(out=out_ap, in0=v_ap, scalar=0.0,
                                     in1=t_ap, op0=MAX, op1=ADD)

        def emit_half_ep(hT, h, sliced=False):
            # per-head half-epilogue: rinv = 1/S via DVE reciprocal, PE
            # partition broadcast, v-half = hT*rinv.  The odd half finishes:
            # hcat = elu(v)+1 = relu(v)+exp(min(v,0)).  `sliced` pipelines
            # the chain in 512-column slices (used for the final head-pair,
            # where this chain gates the whole layer-2 tail).
            rinv = ep1.tile([1, I], F32, tag=f"ri{h % 2}", bufs=1)
            if h % 2 == 0:
                v = ep1.tile([128, I], BF16, tag="v", bufs=2)
                ep_state["v"] = v
                half = slice(0, D)
            else:
                v = ep_state["v"]
                half = slice(D, 128)
            rbp = psB.tile([128, I], F32, tag="rep")
            # HW: a DVE op may read only ONE input from PSUM, so the
            # broadcast row block is staged through SBUF (rbc)
            rbc = ep1.tile([D, I], F32, tag=f"rb{h % 2}", bufs=1)
            t = None
            if h % 2 == 1:
                t = ep1.tile([128, I], BF16, tag="t", bufs=2)
            for hf_ in range(I // 512):
                sl_ = slice(hf_ * 512, (hf_ + 1) * 512)
                if rinv_eng == "act":
                    nc.scalar.activation(rinv[0:1, sl_], hT[D:D + 1, sl_],
                                         ACT_RECIP)
                else:
                    nc.vector.reciprocal(rinv[0:1, sl_], hT[D:D + 1, sl_])
                nc.tensor.matmul(rbp[0:D, sl_], lhsT=onesf_s[0:1, 0:D],
                                 rhs=rinv[0:1, sl_])
                if sliced:
                    _copy_engine(nc, rbc_eng, rbc[:, sl_], rbp[0:D, sl_])
                    nc.vector.tensor_tensor(out=v[half, sl_],
                                            in0=hT[0:D, sl_],
                                            in1=rbc[:, sl_], op=MULT)
                    if h % 2 == 1:
                        nc.vector.tensor_scalar_min(t[:, sl_], v[:, sl_], 0.0)
                        nc.scalar.activation(t[:, sl_], t[:, sl_], ACT_EXP)
                        _stt_hcat(hcatT[:, h // 2, sl_], v[:, sl_], t[:, sl_])
            if not sliced:
                _copy_engine(nc, rbc_eng, rbc[:], rbp[0:D, :])
                nc.vector.tensor_tensor(out=v[half, :], in0=hT[0:D, :],
                                        in1=rbc[:], op=MULT)
                if h % 2 == 1:
                    nc.vector.tensor_scalar_min(t[:], v[:], 0.0)
                    nc.scalar.activation(t[:], t[:], ACT_EXP)
                    _stt_hcat(hcatT[:, h // 2, :], v[:], t[:])

        ph0_done = set()

        def emit_phase0_once(h):
            if h not in ph0_done:
                ph0_done.add(h)
                emit_phase0_head(h)

        def emit_head_prep(h):
            emit_phase0_once(h)
            head_modes = route[h * PAIRS_PER_HEAD:(h + 1) * PAIRS_PER_HEAD]
            need_f1rep = any(m in (3, 4) for m in head_modes)
            need_B = any(m in (5, 7) for m in head_modes)
            f1p = psB.tile([128, I], F32, tag="rep")
            for hf in range(I // 512):
                sl = slice(hf * 512, (hf + 1) * 512)
                nc.tensor.matmul(f1p[:, sl], lhsT=a1rep_s[:, h, :],
                                 rhs=xtl_s[:, sl])
            f1rep_s = B1_s = None
            if need_f1rep:
                f1rep_s = ep2.tile([128, I], BF16, tag="f1rep")
                _copy_engine(nc, f1rep_eng, f1rep_s[:], f1p[:])
            if need_B:
                B1_s = ep2.tile([128, I], BF16, tag="B1")
                nc.scalar.activation(B1_s[:], f1p[:], ACT_EXP, scale=1.0 - ALPHA)
            return f1rep_s, B1_s

        # ---------------- layer 1 ----------------
        # prefetch phase-0 (Wh/fcol) for the first heads: PE + copies run
        # under the startup mask-DMA window (head 0 first)
        for h0_ in range(0, 1 + int(cfg.get("prefetch_heads", 1))):
            emit_phase0_once(h0_)
        pending_ep = None   # deferred half-epilogue (software pipelining)
        preps = emit_head_prep(0)
        for h in range(H):
            head_modes = route[h * PAIRS_PER_HEAD:(h + 1) * PAIRS_PER_HEAD]
            cur = preps
            hT = psC.tile([D + 1, I], F32, tag="acc")
            for jp in range(PAIRS_PER_HEAD):
                if jp == int(cfg.get("ep_defer", 2)) and pending_ep is not None:
                    pending_ep()
                    pending_ep = None
                if jp == int(cfg.get("prep_at", 4)) and h + 1 < H:
                    preps = emit_head_prep(h + 1)
                mode = head_modes[jp]
                subs = []
                for k in range(2):
                    jt = jp * 2 + k
                    subs.append((jt, mb_s[:, jt, :],
                                 fcol[:, h, jt, :], fexp1[:, h, jt, :],
                                 fexp2[:, h, jt, :]))
                p = _emit_pair(nc, work, workp, mode, subs, cur,
                               mb_s[:, jp * 2:jp * 2 + 2, :])
                for k in range(2):
                    jt = jp * 2 + k
                    for hf in range(I // 512):
                        sl = slice(hf * 512, (hf + 1) * 512)
                        nc.tensor.matmul(hT[:, sl],
                                         lhsT=whbuf[:, h, jt, :],
                                         rhs=p[:, k, sl],
                                         start=(jt == 0), stop=(jt == JT - 1))

            pending_ep = (lambda t_=hT, h_=h, s_=(h == H - 1):
                          emit_half_ep(t_, h_, sliced=s_))
        if pending_ep is not None:
            pending_ep()
            pending_ep = None

        # ---------------- layer 2 projection + exchange -------------------
        # HOST permutes the key order per core to [my I queries; partner's I
        # queries], so key tiles jt 0..7 are LOCAL (read straight from
        # wh2loc, no collective round-trip) and only tiles 8..15 need the
        # partner's projection.  The exchange is an AllReduce(add) of the
        # local projection; partner = sum - mine (exact to f32 rounding).
        wh2loc = singles.tile([128, IC, C + 2], F32)
        gin = dram.tile([I, C + 2], F32)
        g1rowp = psB.tile([128, I], F32, tag="rep")
        for ic in range(IC):
            w2p = psA.tile([128, 4, D + 2], F32, tag="ph")
            for kt in range(KT):
                nc.tensor.matmul(
                    w2p[:, 0, 0:C + 2],
                    lhsT=hcatT[:, kt, ic * 128:(ic + 1) * 128],
                    rhs=woext_s[:, kt, :],
                    start=(kt == 0), stop=False)
            nc.tensor.matmul(w2p[:, 0, 0:C + 2], lhsT=onesf_s[0:1, :],
                             rhs=wcorr_s[:], start=False, stop=True)
            nc.vector.tensor_copy(out=wh2loc[:, ic, :], in_=w2p[:, 0, 0:C + 2])
            nc.tensor.transpose(g1rowp[0:1, ic * 128:(ic + 1) * 128],
                                in_=wh2loc[:, ic, 0:1], identity=ident_s[:])
        gsum = dram.tile([I, C + 2], F32)
        # exchange pipelined in two halves: gin-half DMA -> AllReduce-half
        # (or local fake copy) -> wh2sum-half DMA
        for hf in range(2):
            ghalf = slice(hf * 512, (hf + 1) * 512)
            nc.sync.dma_start(
                out=gin[ghalf, :].rearrange("(ic p) c -> p ic c", p=128),
                in_=wh2loc[:, hf * 4:(hf + 1) * 4, :])
            if with_collective:
                nc.gpsimd.collective_compute(
                    "AllReduce", mybir.AluOpType.add,
                    replica_groups=REPLICA_GROUPS,
                    ins=[gin[ghalf, :].opt()], outs=[gsum[ghalf, :].opt()])
            else:  # timing-model variant: fake the exchange, local copy
                nc.sync.dma_start(out=gsum[ghalf, :], in_=gin[ghalf, :])

        # g1 row (local queries) -> replicated [128, I]
        g1row_s = epL2.tile([1, I], BF16, tag="g1row")
        nc.scalar.activation(g1row_s[:], g1rowp[0:1, :], ACT_COPY)
        g1rp = psB.tile([128, I], F32, tag="rep")
        for hf in range(I // 512):
            sl = slice(hf * 512, (hf + 1) * 512)
            nc.tensor.matmul(g1rp[:, sl], lhsT=ones_s[0:1, :],
                             rhs=g1row_s[0:1, sl])
        l2_modes = route[H * PAIRS_PER_HEAD:]
        g1rep_s = B1L2 = None
        if any(m in (3, 4) for m in l2_modes):
            g1rep_s = singles.tile([128, I], BF16)
            nc.vector.tensor_copy(out=g1rep_s[:], in_=g1rp[:])
        if any(m in (5, 7) for m in l2_modes):
            B1L2 = singles.tile([128, I], BF16)
            nc.scalar.activation(B1L2[:], g1rp[:], ACT_EXP, scale=1.0 - ALPHA)

        # key-side rows: [g1, g2, Wh2(32)] f32, bf16 for the PV lhsT.
        # Local tiles (jt 0..7) come straight from wh2loc; remote tiles
        # (jt 8..15) from the AllReduce sum minus the local projection.
        JH = JT // 2
        wh2gr = singles.tile([128, JT, C + 3], BF16)
        nc.gpsimd.memset(wh2gr[:, :, C + 2:C + 3], 1.0)
        its1 = singles.tile([128, JT, 1], F32)
        its2 = singles.tile([128, JT, 1], F32)
        for jg in range(JH // 4):
            s4 = slice(jg * 4, (jg + 1) * 4)
            nc.gpsimd.tensor_copy(out=wh2gr[:, s4, 0:C + 2],
                                  in_=wh2loc[:, s4, :])
            nc.scalar.activation(its1[:, s4, :], wh2loc[:, s4, 1:2], ACT_EXP)
            nc.scalar.activation(its2[:, s4, :], wh2loc[:, s4, 1:2], ACT_EXP,
                                 scale=ALPHA)
        wh2sum = singles.tile([128, JH, C + 2], F32)
        wh2rem = singles.tile([128, JH, C + 2], F32)
        for jg in range(JH // 4):
            s4 = slice(jg * 4, (jg + 1) * 4)
            s4r = slice(JH + jg * 4, JH + (jg + 1) * 4)
            nc.sync.dma_start(
                out=wh2sum[:, s4, :],
                in_=gsum[jg * 512:(jg + 1) * 512, :].rearrange(
                    "(jt p) c -> p jt c", p=128))
            nc.vector.tensor_tensor(out=wh2rem[:, s4, :],
                                    in0=wh2sum[:, s4, :], in1=wh2loc[:, s4, :],
                                    op=mybir.AluOpType.subtract)
            nc.gpsimd.tensor_copy(out=wh2gr[:, s4r, 0:C + 2],
                                  in_=wh2rem[:, s4, :])
            nc.scalar.activation(its1[:, s4r, :], wh2rem[:, s4, 1:2], ACT_EXP)
            nc.scalar.activation(its2[:, s4r, :], wh2rem[:, s4, 1:2], ACT_EXP,
                                 scale=ALPHA)

        # ---------------- layer 2 attention ----------------
        o2T = psC.tile([D + 1, I], F32, tag="acc")
        for jp in range(PAIRS_PER_HEAD):
            mode = l2_modes[jp]
            subs = []
            for k in range(2):
                jt = jp * 2 + k
                f2c = (wh2loc[:, jt, 1:2] if jt < JH
                       else wh2rem[:, jt - JH, 1:2])
                subs.append((jt, mb_s[:, jt, :],
                             f2c, its1[:, jt, :],
                             its2[:, jt, :]))
            p = _emit_pair(nc, work, workp, mode, subs,
                           (g1rep_s, B1L2), mb_s[:, jp * 2:jp * 2 + 2, :])
            for k in range(2):
                jt = jp * 2 + k
                for hf in range(I // 512):
                    sl = slice(hf * 512, (hf + 1) * 512)
                    nc.tensor.matmul(o2T[0:C + 1, sl],
                                     lhsT=wh2gr[:, jt, 2:C + 3],
                                     rhs=p[:, k, sl],
                                     start=(jt == 0), stop=(jt == JT - 1))

        # ---------------- finalize (transposed: per-query reciprocal) -----
        # per 512-slice: rinv2 (DVE recip) -> PE row-broadcast -> oT =
        # o2T * rbc2 (PSUM x SBUF) -> per-128 transposes -> out DMA halves
        rinv2 = epL2.tile([1, I], F32, tag="ri2")
        oT_s = epL2.tile([C, I], F32, tag="oT")
        ofs_all = epL2.tile([128, IC, C], F32, tag="ofs_all")
        rbc2p = psB.tile([128, I], F32, tag="rep")
        rbc2_s = epL2.tile([C, I], F32, tag="rbc2")
        for hf in range(I // 512):
            sl = slice(hf * 512, (hf + 1) * 512)
            nc.vector.reciprocal(rinv2[0:1, sl], o2T[C:C + 1, sl])
            nc.tensor.matmul(rbc2p[0:C, sl], lhsT=onesf_s[0:1, 0:C],
                             rhs=rinv2[0:1, sl])
            _copy_engine(nc, rbc_eng, rbc2_s[:, sl], rbc2p[0:C, sl])
            nc.vector.tensor_tensor(out=oT_s[:, sl], in0=o2T[0:C, sl],
                                    in1=rbc2_s[:, sl], op=MULT)
            for k in range(hf * 4, hf * 4 + 4):
                ofp = psA.tile([128, 4, D + 2], F32, tag="ph")
                nc.tensor.transpose(ofp[:, 0, 0:C],
                                    in_=oT_s[:, k * 128:(k + 1) * 128],
                                    identity=ident_s[0:C, 0:C])
                nc.vector.tensor_copy(out=ofs_all[:, k, :],
                                      in_=ofp[:, 0, 0:C])
            nc.sync.dma_start(
                out=outp_d.ap()[hf * 512:(hf + 1) * 512, :].rearrange(
                    "(k p) c -> p k c", p=128),
                in_=ofs_all[:, hf * 4:hf * 4 + 4, :])


# --------------------------------------------------------------------------
# host side
# --------------------------------------------------------------------------

def shard_inputs(x, adj, W, a1, a2, Wo, ao1, ao2):
    x = np.asarray(x, np.float32)
    adj = np.asarray(adj)
    W = np.asarray(W, np.float32)
    a1 = np.asarray(a1, np.float32)
    a2 = np.asarray(a2, np.float32)
    Wo = np.asarray(Wo, np.float32)
    ao1 = np.asarray(ao1, np.float32)
    ao2 = np.asarray(ao2, np.float32)
    BF = ml_dtypes.bfloat16

    wvec1 = np.einsum("hfd,hd->hf", W, a1)          # [H, F]
    wvec2 = np.einsum("hfd,hd->hf", W, a2)
    wext = np.concatenate([W, wvec1[:, :, None], wvec2[:, :, None]],
                          axis=2).astype(BF)
    a1rep = np.repeat(wvec1[:, :, None], 128, axis=2).astype(BF)
    wo1 = Wo @ ao1                                   # [512]
    wo2 = Wo @ ao2
    woflat = np.concatenate([wo1[:, None], wo2[:, None], Wo], 1)  # [512, 34]
    woext = woflat.reshape(KT, 128, C + 2).astype(BF)
    wcorr = (-woflat.sum(0))[None, :].astype(np.float32)
    ident = np.eye(128, dtype=np.float32)

    in_maps = []
    for c in range(N_CORES):
        b, half = c // 2, c % 2
        i0 = half * I
        # Per-core key order [my I queries; partner's I queries]: key tiles
        # jt 0..7 are then LOCAL in layer 2 (see kernel L2 exchange).
        # Attention is key-order invariant as long as xt and the mask rows
        # are permuted consistently.
        perm = np.r_[i0:i0 + I, (I - i0):(I - i0) + I]
        xt = np.ascontiguousarray(x[b].T[:, perm]).astype(BF)   # [F, N]
        xtl = np.ascontiguousarray(xt[:, 0:I])
        adjt = adj[b, i0:i0 + I, :].T[perm, :]       # [N, I] = (j, i)
        mb = np.where(adjt > 0, np.float32(1.0), np.float32(0.0))
        mb = np.ascontiguousarray(mb.reshape(JT, 128, I)).astype(BF)
        in_maps.append({
            "xt": xt, "xtl": xtl, "mb": mb, "wext": wext,
            "a1rep": a1rep, "woext": woext, "wcorr": wcorr, "ident": ident,
        })
    return in_maps


# Engine routing: multiplicative-mask pipelines balanced across
# DVE (P7) / DVE+GPS (P5) / ACT+DVE (P3) / ACT+GPS (P4) by cost-model
# (TimelineSim) hill-climb.
DEFAULT_CFG = {"ep_defer": 3, "f1rep_eng": "act", "rbc_eng": "act",
               "prep_at": 4, "p4": 0, "p7": 20, "p5": 24,
               "l2route": [7, 5, 7, 7, 5, 7, 7, 7],
               "pbufs": 8, "wbufs": 6, "qbufs": 4, "prefetch_heads": 1}

_CACHE = {}


def _program():
    if "nc" not in _CACHE:
        _CACHE["nc"] = build_program(with_collective=True, cfg=DEFAULT_CFG)
    return _CACHE["nc"]


def kernel(**inputs):
    nc = _program()
    in_maps = shard_inputs(**inputs)
    res = run_bass_kernel_spmd(nc, in_maps, list(range(N_CORES)))
    _CACHE["last_results"] = res
    out = np.empty((B, N, C), np.float32)
    for c in range(N_CORES):
        b, half = c // 2, c % 2
        out[b, half * I:(half + 1) * I, :] = res.results[c]["outp"]
    return out

